# revision 1
# baseline (speedup 1.0000x reference)
"""MoE head kernel for Trainium2 (8 NeuronCores, data-parallel over batch).

Computes, per the reference nn.Module:
  w      = softmax(cos_sim(z_cat, mu_cat) / tau)          # gate  [B, E]
  xhat   = LayerNorm(feat)  (no affine applied yet)
  x_e    = xhat * gamma_e + beta_e                         # per-expert affine
  h_e    = relu(x_e @ W1_e + b1_e)
  l_e    = h_e @ W2_e + b2_e
  logits = sum_e w[:, e] * l_e                             # [B, C]
returns (logits, w).

Sharding: batch B=16384 split 8 ways (2048 rows/core); all params replicated.
No collectives. Everything computed on-device; outputs gathered on host.

Layout strategy per core:
  - LN in [B, D] layout (rows on partitions), then PE-transpose to
    xhatT [D, B] so the D-contraction matmul has D on partitions.
  - mm1: out hT [H-tile(128), Bchunk(512)] = W1_strip.T @ xhatT, accumulated
    over 8 K-tiles in PSUM; fused bias+relu on ScalarE into SBUF.
  - mm2: out lT [8, Bchunk] = W2_strip.T @ hT, accumulated over 16 H-tiles
    in PSUM (one bank per B-chunk, 4 chunks live at once).
  - lT + b2 -> PE-transpose back to [B-tile, 8] -> scale by gate column
    w[:, e] (a per-partition scalar in this layout) -> accumulate logits.
Matmul operands are bitcast to float32r (full-rate fp32 on the PE when the
moving free dim >= 256; mm1 rhs is 512 wide).
"""

import numpy as np
from contextlib import ExitStack

import concourse.bass as bass
import concourse.mybir as mybir
import concourse.tile as tile
from concourse import bacc
from concourse.masks import make_identity
from concourse.bass_utils import run_bass_kernel_spmd

# Problem shapes (hardcoded per contract).
B, D, H, E, DZ = 16384, 1024, 2048, 8, 256
NCORES = 8
BS = B // NCORES            # rows per core = 2048
CHUNK = 512                 # batch chunk for matmul free dim
NCH = BS // CHUNK           # 4
BT = BS // 128              # 16 partition tiles of batch
KD = D // 128               # 8 K-tiles for mm1
MH = H // 128               # 16 M-tiles of hidden
KZ = DZ // 128              # 2 K-tiles for the gate matmul
LN_EPS = 1e-5

F32 = mybir.dt.float32
AF = mybir.ActivationFunctionType
ALU = mybir.AluOpType
AX = mybir.AxisListType


def _build(tau: float, affine: bool, mm_dt=mybir.dt.float32r):
    nc = bacc.Bacc(None, target_bir_lowering=False, name="moe_head")

    feat = nc.dram_tensor("feat", [BS, D], F32, kind="ExternalInput")
    z = nc.dram_tensor("z", [BS, DZ], F32, kind="ExternalInput")
    mu = nc.dram_tensor("mu", [E, DZ], F32, kind="ExternalInput")
    w1 = nc.dram_tensor("w1", [E, D, H], mm_dt, kind="ExternalInput")
    b1 = nc.dram_tensor("b1", [E, H], F32, kind="ExternalInput")
    w2 = nc.dram_tensor("w2", [E, H, E], mm_dt, kind="ExternalInput")
    b2 = nc.dram_tensor("b2", [E, E], F32, kind="ExternalInput")
    if affine:
        gam = nc.dram_tensor("gam", [E, D], F32, kind="ExternalInput")
        bet = nc.dram_tensor("bet", [E, D], F32, kind="ExternalInput")
    logits_o = nc.dram_tensor("logits", [BS, E], F32, kind="ExternalOutput")
    w_o = nc.dram_tensor("w", [BS, E], F32, kind="ExternalOutput")

    inv_tau = 1.0 / tau

    with tile.TileContext(nc) as tc, ExitStack() as ctx:
        persist = ctx.enter_context(tc.tile_pool(name="persist", bufs=1))
        lnpool = ctx.enter_context(tc.tile_pool(name="ln", bufs=3))
        statp = ctx.enter_context(tc.tile_pool(name="stat", bufs=4))
        wpool = ctx.enter_context(tc.tile_pool(name="w1s", bufs=3))
        epool = ctx.enter_context(tc.tile_pool(name="eparam", bufs=2))
        hpool = ctx.enter_context(tc.tile_pool(name="h", bufs=6))
        spool = ctx.enter_context(tc.tile_pool(name="small", bufs=3))
        psA = ctx.enter_context(tc.tile_pool(name="psA", bufs=2, space="PSUM"))
        psB = ctx.enter_context(tc.tile_pool(name="psB", bufs=4, space="PSUM"))
        psC = ctx.enter_context(tc.tile_pool(name="psC", bufs=2, space="PSUM"))

        # Persistent SBUF tensors.
        # xhatT split per B-chunk so the expert loop can start on chunk 0
        # while LN/transpose still runs on later chunks.
        xhatT_c = [persist.tile([128, KD, CHUNK], mm_dt, name=f"xhatT{c}")
                   for c in range(NCH)]
        znT = persist.tile([128, KZ, BS], F32)        # normalized z, transposed
        munT = persist.tile([128, KZ, E], F32)        # normalized mu, transposed
        w_sb = persist.tile([128, BT, E], F32)        # gate weights [B, E]
        acc = persist.tile([128, BT, E], F32)         # logits accumulator [B, C]
        ident = persist.tile([128, 128], F32)
        # b2 columns replicated at partition groups 0/32/64/96 — one copy per
        # mm2 col-group band (band j = B-chunk j's expert logits).
        b2T4 = persist.tile([128, E], F32)
        eps_sb = persist.tile([128, 1], F32)
        if affine:
            gamT = persist.tile([128, KD, E], F32)
            betT = persist.tile([128, KD, E], F32)
            x_eT = persist.tile([128, KD, BS], mm_dt)  # per-expert affine input

        make_identity(nc, ident)
        nc.vector.memset(acc[:], 0.0)
        nc.vector.memset(eps_sb[:], LN_EPS)
        with nc.allow_non_contiguous_dma(reason="tiny strided param loads"):
            for j in range(NCH):
                nc.sync.dma_start(
                    b2T4[32 * j:32 * j + E, :], b2.rearrange("e c -> c e"))
            if affine:
                nc.sync.dma_start(
                    gamT[:], gam.rearrange("e (ko ki) -> ki ko e", ki=128))
                nc.sync.dma_start(
                    betT[:], bet.rearrange("e (ko ki) -> ki ko e", ki=128))

        # ---------------- Phase 0a: gate ----------------
        # mu: normalize rows of [E, DZ], transpose to munT.
        mu_sb = spool.tile([E, DZ], F32, tag="mu")
        nc.sync.dma_start(mu_sb[:], mu[:, :])
        musq = spool.tile([E, DZ], F32, tag="musq")
        muss = statp.tile([E, 1], F32, tag="muss")
        nc.scalar.activation(musq, mu_sb, AF.Square, accum_out=muss)
        mustd = statp.tile([E, 1], F32, tag="mustd")
        nc.scalar.activation(mustd, muss, AF.Sqrt)
        murn = statp.tile([E, 1], F32, tag="murn")
        nc.vector.reciprocal(murn, mustd)
        mu_n = spool.tile([E, DZ], F32, tag="mun")
        nc.vector.tensor_scalar_mul(mu_n[:], mu_sb[:], murn)
        for kz in range(KZ):
            pst = psC.tile([128, 128], F32, tag="tp")
            nc.tensor.transpose(
                pst[:, :E], mu_n[:, kz * 128:(kz + 1) * 128], ident[:E, :E])
            nc.vector.tensor_copy(munT[:, kz, :], pst[:, :E])

        # z: normalize rows tile-by-tile, transpose into znT.
        for bt in range(BT):
            bsl = slice(bt * 128, (bt + 1) * 128)
            zt = lnpool.tile([128, DZ], F32, tag="zt")
            nc.sync.dma_start(zt[:], z[bsl, :])
            zsq = lnpool.tile([128, DZ], F32, tag="zsq")
            zss = statp.tile([128, 1], F32, tag="zss")
            nc.scalar.activation(zsq, zt, AF.Square, accum_out=zss)
            zstd = statp.tile([128, 1], F32, tag="zstd")
            nc.scalar.activation(zstd, zss, AF.Sqrt)
            zrn = statp.tile([128, 1], F32, tag="zrn")
            nc.vector.reciprocal(zrn, zstd)
            zn = lnpool.tile([128, DZ], F32, tag="zn")
            nc.vector.tensor_scalar_mul(zn[:], zt[:], zrn)
            for kz in range(KZ):
                pst = psC.tile([128, 128], F32, tag="tp")
                nc.tensor.transpose(
                    pst[:], zn[:, kz * 128:(kz + 1) * 128], ident[:])
                nc.vector.tensor_copy(znT[:, kz, bsl], pst[:])

        # sims + softmax per batch tile -> w_sb.
        for bt in range(BT):
            bsl = slice(bt * 128, (bt + 1) * 128)
            ps = psC.tile([128, E], F32, tag="tp")
            for kz in range(KZ):
                nc.tensor.matmul(
                    ps[:], znT[:, kz, bsl], munT[:, kz, :],
                    start=(kz == 0), stop=(kz == KZ - 1))
            mx = statp.tile([128, 1], F32, tag="mx")
            nc.vector.reduce_max(mx, ps[:], axis=AX.X)
            nb = statp.tile([128, 1], F32, tag="nb")
            nc.vector.tensor_scalar_mul(nb, mx, -inv_tau)
            ex = spool.tile([128, E], F32, tag="ex")
            nc.scalar.activation(ex[:], ps[:], AF.Exp, bias=nb, scale=inv_tau)
            sm = statp.tile([128, 1], F32, tag="sm")
            nc.vector.reduce_sum(sm, ex[:], axis=AX.X)
            rsm = statp.tile([128, 1], F32, tag="rsm")
            nc.vector.reciprocal(rsm, sm)
            nc.vector.tensor_scalar_mul(w_sb[:, bt, :], ex[:], rsm)

        # ---------------- Phase 0b: LayerNorm + transpose ----------------
        for bt in range(BT):
            bsl = slice(bt * 128, (bt + 1) * 128)
            ft = lnpool.tile([128, D], F32, tag="ft")
            nc.sync.dma_start(ft[:], feat[bsl, :])
            s1 = statp.tile([128, 1], F32, tag="s1")
            nc.vector.reduce_sum(s1, ft[:], axis=AX.X)
            nm = statp.tile([128, 1], F32, tag="nm")
            nc.vector.tensor_scalar_mul(nm, s1, -1.0 / D)
            xc = lnpool.tile([128, D], F32, tag="xc")
            nc.vector.tensor_scalar_add(xc[:], ft[:], nm)
            sq = lnpool.tile([128, D], F32, tag="sq")
            ss = statp.tile([128, 1], F32, tag="ss")
            nc.scalar.activation(sq, xc[:], AF.Square, accum_out=ss)
            std = statp.tile([128, 1], F32, tag="std")
            nc.scalar.activation(std, ss, AF.Sqrt, bias=eps_sb[:], scale=1.0 / D)
            rs = statp.tile([128, 1], F32, tag="rs")
            nc.vector.reciprocal(rs, std)
            xh = lnpool.tile([128, D], F32, tag="xh")
            nc.vector.tensor_scalar_mul(xh[:], xc[:], rs)
            c, lo = divmod(bt * 128, CHUNK)
            for kd in range(KD):
                pst = psC.tile([128, 128], F32, tag="tp")
                nc.tensor.transpose(
                    pst[:], xh[:, kd * 128:(kd + 1) * 128], ident[:])
                nc.vector.tensor_copy(
                    xhatT_c[c][:, kd, lo:lo + 128], pst[:])

        # ---------------- Phase 1: experts ----------------
        for e in range(E):
            w2sb = epool.tile([128, MH, E], mm_dt, tag="w2sb")
            b1sb = epool.tile([128, MH], F32, tag="b1sb")
            with nc.allow_non_contiguous_dma(reason="per-expert param loads"):
                nc.sync.dma_start(
                    w2sb[:], w2[e].rearrange("(ko ki) c -> ki ko c", ki=128))
                nc.sync.dma_start(
                    b1sb[:], b1[e].rearrange("(mo mi) -> mi mo", mi=128))

            if affine:
                for kd in range(KD):
                    for c in range(NCH):
                        nc.scalar.activation(
                            x_eT[:, kd, c * CHUNK:(c + 1) * CHUNK],
                            xhatT_c[c][:, kd, :], AF.Identity,
                            bias=betT[:, kd, e:e + 1],
                            scale=gamT[:, kd, e:e + 1])

            def rhs_for(k, c):
                if affine:
                    return x_eT[:, k, c * CHUNK:(c + 1) * CHUNK]
                return xhatT_c[c][:, k, :]

            ps2 = [psB.tile([E, CHUNK], F32, tag="ps2", name=f"ps2_{e}_{c}")
                   for c in range(NCH)]

            for m in range(MH):
                strip = wpool.tile([128, KD, 128], mm_dt, tag="w1s")
                nc.sync.dma_start(
                    strip[:],
                    w1[e, :, m * 128:(m + 1) * 128].rearrange(
                        "(ko ki) m -> ki ko m", ki=128))
                for c in range(NCH):
                    ps1 = psA.tile([128, CHUNK], F32, tag="ps1")
                    for k in range(KD):
                        nc.tensor.matmul(
                            ps1[:],
                            strip[:, k, :],
                            rhs_for(k, c),
                            start=(k == 0), stop=(k == KD - 1))
                    hsb = hpool.tile([128, CHUNK], mm_dt, tag="h")
                    nc.scalar.activation(
                        hsb[:], ps1[:], AF.Relu, bias=b1sb[:, m:m + 1])
                    nc.tensor.matmul(
                        ps2[c][:],
                        w2sb[:, m, :],
                        hsb[:],
                        start=(m == 0), stop=(m == MH - 1))

            # Drain: add b2, transpose back to [B, C], weight by gate, accum.
            for c in range(NCH):
                lsb = spool.tile([E, CHUNK], F32, tag="lsb")
                nc.scalar.activation(
                    lsb[:], ps2[c][:], AF.Identity, bias=b2T4[:E, e:e + 1])
                for sub in range(CHUNK // 128):
                    bt = c * (CHUNK // 128) + sub
                    pst = psC.tile([128, E], F32, tag="tp")
                    nc.tensor.transpose(
                        pst[:], lsb[:, sub * 128:(sub + 1) * 128],
                        ident[:E, :E])
                    tmp = spool.tile([128, E], F32, tag="ltmp")
                    nc.vector.tensor_scalar_mul(
                        tmp[:], pst[:], w_sb[:, bt, e:e + 1])
                    nc.vector.tensor_tensor(
                        acc[:, bt, :], acc[:, bt, :], tmp[:], ALU.add)

        # ---------------- Outputs ----------------
        nc.sync.dma_start(
            logits_o.rearrange("(bo bi) c -> bi bo c", bi=128), acc[:])
        nc.sync.dma_start(
            w_o.rearrange("(bo bi) c -> bi bo c", bi=128), w_sb[:])

    nc.compile()
    return nc


_CACHE = {}


def kernel(**inputs):
    feat = np.ascontiguousarray(inputs["feat"], dtype=np.float32)
    z_cat = np.ascontiguousarray(inputs["z_cat"], dtype=np.float32)
    mu_cat = np.ascontiguousarray(inputs["mu_cat"], dtype=np.float32)
    ln_gamma = np.asarray(inputs["ln_gamma"], dtype=np.float32)
    ln_beta = np.asarray(inputs["ln_beta"], dtype=np.float32)
    W1 = np.ascontiguousarray(inputs["W1"], dtype=np.float32)
    b1 = np.ascontiguousarray(inputs["b1"], dtype=np.float32)
    W2 = np.ascontiguousarray(inputs["W2"], dtype=np.float32)
    b2 = np.ascontiguousarray(inputs["b2"], dtype=np.float32)
    tau = max(1e-6, float(inputs["tau_gate"]))

    affine = not (
        np.all(ln_gamma == 1.0) and np.all(ln_beta == 0.0))

    key = (tau, affine)
    if key not in _CACHE:
        _CACHE[key] = _build(tau, affine)
    nc = _CACHE[key]

    in_maps = []
    for c in range(NCORES):
        rs = slice(c * BS, (c + 1) * BS)
        m = {
            "feat": feat[rs],
            "z": z_cat[rs],
            "mu": mu_cat,
            "w1": W1,
            "b1": b1,
            "w2": W2,
            "b2": b2,
        }
        if affine:
            m["gam"] = ln_gamma
            m["bet"] = ln_beta
        in_maps.append(m)

    res = run_bass_kernel_spmd(nc, in_maps, core_ids=list(range(NCORES)))
    outs = res.results
    logits = np.concatenate([o["logits"] for o in outs], axis=0)
    w = np.concatenate([o["w"] for o in outs], axis=0)
    return logits.astype(np.float32), w.astype(np.float32)



# revision 8
# speedup vs baseline: 1.0246x; 1.0246x over previous
"""MoE head kernel for Trainium2 (8 NeuronCores, data-parallel over batch).

Per the reference nn.Module:
  w      = softmax(cos_sim(z_cat, mu_cat) / tau)          # gate  [B, E]
  xhat   = LayerNorm(feat)                                 # affine folded into W1/b1
  h_e    = relu(xhat @ W1_e + b1_e)
  l_e    = h_e @ W2_e + b2_e
  logits = sum_e w[:, e] * l_e                             # [B, C]
returns (logits, w).

The LN affine (gamma/beta) is folded into W1/b1 on the host (exact:
x_e @ W1 = xhat @ (gamma*W1) + beta @ W1), so the device kernel has a
single shared xhat for all experts.

Sharding: batch B=16384 split 8 ways (2048 rows/core); params replicated.

Per-core layout (all matmul operands bf16 -> FWL weight loads, 1 cyc/row):
  - LN in [B, D] layout with a fused (x+(-mean))*rstd tensor_scalar, then
    transposed to xhatT [D, B] via regular matmuls against an identity
    (much faster than PE transpose-mode and counts as PE-busy for HAM).
  - mm1: hT [128, 512] = W1_strip.T @ xhatT accumulated over 8 K-tiles in
    PSUM; relu+bias on ScalarE -> bf16; mm2 accumulates lT [8, 512] over
    16 H-tiles in a second PSUM bank group.
  - Gate (cos-sim softmax) is emitted AFTER expert 0's matmul loop so its
    PE work hides behind mm1; it produces w (for output), wT, and a
    partition-broadcast wB[c, e, b] = w[b, e] via tiny selector matmuls.
  - Drain per expert: logitsT += ps2 * wB[:, e, :] on VectorE only (no
    per-expert transposes). b2 is pre-accumulated into logitsT via a
    single b2.T @ wT matmul per chunk.
  - Final: 16 identity-matmul transposes back to [B, C], DMA out.
"""

import numpy as np
from contextlib import ExitStack

import ml_dtypes

import concourse.bass as bass
import concourse.mybir as mybir
import concourse.tile as tile
from concourse import bacc
from concourse.masks import make_identity
from concourse.bass_utils import run_bass_kernel_spmd

# Problem shapes (hardcoded per contract).
B, D, H, E, DZ = 16384, 1024, 2048, 8, 256
NCORES = 8
BS = B // NCORES            # rows per core = 2048
CHUNK = 512                 # batch chunk (PSUM bank = 512 fp32)
NCH = BS // CHUNK           # 4
BT = BS // 128              # 16 partition tiles of batch
KD = D // 128               # 8 K-tiles for mm1
MH = H // 128               # 16 M-tiles of hidden
KZ = DZ // 128              # 2 K-tiles for the gate matmul
LN_EPS = 1e-5

F32 = mybir.dt.float32
BF16 = mybir.dt.bfloat16
NPBF = ml_dtypes.bfloat16
AF = mybir.ActivationFunctionType
ALU = mybir.AluOpType
AX = mybir.AxisListType


def _build(tau: float):
    nc = bacc.Bacc(None, target_bir_lowering=False, name="moe_head")

    feat = nc.dram_tensor("feat", [BS, D], F32, kind="ExternalInput")
    z = nc.dram_tensor("z", [BS, DZ], F32, kind="ExternalInput")
    mu = nc.dram_tensor("mu", [E, DZ], F32, kind="ExternalInput")
    # w1 host layout: [e, mt, ki, ko, mi] so each strip DMA is contiguous.
    w1 = nc.dram_tensor("w1", [E, MH, 128, KD, 128], BF16, kind="ExternalInput")
    # w2 host layout: [e, ki, ko, c]
    w2 = nc.dram_tensor("w2", [E, 128, MH, E], BF16, kind="ExternalInput")
    # b1 host layout: [e, mi, mo]
    b1 = nc.dram_tensor("b1", [E, 128, MH], F32, kind="ExternalInput")
    b2 = nc.dram_tensor("b2", [E, E], BF16, kind="ExternalInput")
    sel_d = nc.dram_tensor("sel", [E, E * E], BF16, kind="ExternalInput")
    logits_o = nc.dram_tensor("logits", [BS, E], F32, kind="ExternalOutput")
    w_o = nc.dram_tensor("w", [BS, E], F32, kind="ExternalOutput")

    inv_tau = 1.0 / tau

    with tile.TileContext(nc) as tc, ExitStack() as ctx:
        persist = ctx.enter_context(tc.tile_pool(name="persist", bufs=1))
        lnpool = ctx.enter_context(tc.tile_pool(name="ln", bufs=3))
        statp = ctx.enter_context(tc.tile_pool(name="stat", bufs=4))
        wpool = ctx.enter_context(tc.tile_pool(name="w1s", bufs=3))
        epool = ctx.enter_context(tc.tile_pool(name="eparam", bufs=2))
        hpool = ctx.enter_context(tc.tile_pool(name="h", bufs=6))
        spool = ctx.enter_context(tc.tile_pool(name="small", bufs=3))
        psA = ctx.enter_context(tc.tile_pool(name="psA", bufs=2, space="PSUM"))
        psB = ctx.enter_context(tc.tile_pool(name="psB", bufs=4, space="PSUM"))
        psT = ctx.enter_context(tc.tile_pool(name="psT", bufs=2, space="PSUM"))

        # ---- persistent SBUF ----
        xhatT_c = [persist.tile([128, KD, CHUNK], BF16, name=f"xhatT{c}")
                   for c in range(NCH)]
        znT = persist.tile([128, KZ, BS], BF16)
        munT = persist.tile([128, KZ, E], BF16)
        wT = persist.tile([E, BS], BF16)          # gate weights, transposed
        wB = persist.tile([E, E, BS], BF16)       # w[b, e] bcast to C partitions
        w_sb = persist.tile([128, BT, E], F32)    # gate weights [B, E]
        accT = persist.tile([E, BS], F32)         # logitsT accumulator
        acc_out = persist.tile([128, BT, E], F32)
        identbf = persist.tile([128, 128], BF16)
        ident8b = persist.tile([E, E], BF16)
        ident8f = persist.tile([E, E], F32)
        sel = persist.tile([E, E * E], BF16)      # sel[:, 8e:8e+8]: row e ones
        b2s = persist.tile([E, E], BF16)
        zall = persist.tile([128, BT, DZ], F32)
        mu_sb = persist.tile([E, DZ], F32)
        mun_b = persist.tile([E, DZ], BF16)
        eps_sb = persist.tile([128, 1], F32)

        make_identity(nc, identbf)
        make_identity(nc, ident8b)
        make_identity(nc, ident8f)
        nc.vector.memset(eps_sb[:], LN_EPS)

        # activations on sync queue; gate inputs on scalar queue;
        # weights on gpsimd queue (independent DMA streams).
        nc.scalar.dma_start(zall[:], z.rearrange("(bo bi) d -> bi bo d", bi=128))
        nc.scalar.dma_start(mu_sb[:], mu[:, :])
        nc.gpsimd.dma_start(b2s[:], b2[:, :])
        nc.gpsimd.dma_start(sel[:], sel_d[:, :])

        # ---------------- Phase A: LayerNorm + transpose ----------------
        for bt in range(BT):
            bsl = slice(bt * 128, (bt + 1) * 128)
            ft = lnpool.tile([128, D], F32, tag="ft")
            nc.sync.dma_start(ft[:], feat[bsl, :])
            s1 = statp.tile([128, 1], F32, tag="s1")
            nc.vector.tensor_reduce(s1, ft[:], AX.X, ALU.add)
            sq = lnpool.tile([128, D], F32, tag="sq")
            ss = statp.tile([128, 1], F32, tag="ss")
            nc.scalar.activation(sq, ft[:], AF.Square, accum_out=ss)
            nm = statp.tile([128, 1], F32, tag="nm")
            nc.vector.tensor_scalar_mul(nm, s1, -1.0 / D)
            ms = statp.tile([128, 1], F32, tag="ms")
            nc.vector.tensor_tensor(ms, nm, nm, ALU.mult)
            # var = ss/D - mean^2
            vv = statp.tile([128, 1], F32, tag="vv")
            nc.vector.scalar_tensor_tensor(vv, ss, 1.0 / D, ms, ALU.mult,
                                           ALU.subtract)
            std = statp.tile([128, 1], F32, tag="std")
            nc.scalar.activation(std, vv, AF.Sqrt, bias=eps_sb[:])
            rs = statp.tile([128, 1], F32, tag="rs")
            nc.vector.reciprocal(rs, std)
            xh = lnpool.tile([128, D], BF16, tag="xh")
            nc.vector.tensor_scalar(xh[:], ft[:], nm, rs, ALU.add, ALU.mult)
            c, lo = divmod(bt * 128, CHUNK)
            for g in range(2):
                tp = psT.tile([128, 4, 128], F32, tag="tp")
                for j in range(4):
                    kd = g * 4 + j
                    nc.tensor.matmul(
                        tp[:, j, :], xh[:, kd * 128:(kd + 1) * 128],
                        identbf[:], start=True, stop=True)
                dst = xhatT_c[c][:, g * 4:(g + 1) * 4, lo:lo + 128]
                if g == 0:
                    nc.scalar.activation(dst, tp[:], AF.Copy)
                else:
                    nc.vector.tensor_copy(dst, tp[:])

        # ---------------- expert emission helpers ----------------
        def emit_expert_mm(e, w2sb, b1sb, ps2):
            for mt in range(MH):
                strip = wpool.tile([128, KD, 128], BF16, tag="w1s")
                nc.gpsimd.dma_start(strip[:], w1[e, mt])
                for c in range(NCH):
                    ps1 = psA.tile([128, CHUNK], F32, tag="ps1")
                    for k in range(KD):
                        nc.tensor.matmul(
                            ps1[:], strip[:, k, :], xhatT_c[c][:, k, :],
                            start=(k == 0), stop=(k == KD - 1))
                    h = hpool.tile([128, CHUNK], BF16, tag="h")
                    nc.scalar.activation(h[:], ps1[:], AF.Relu,
                                         bias=b1sb[:, mt:mt + 1])
                    nc.tensor.matmul(
                        ps2[c][:], w2sb[:, mt, :], h[:],
                        start=(mt == 0), stop=(mt == MH - 1))

        def emit_expert_drain(e, ps2):
            for c in range(NCH):
                csl = slice(c * CHUNK, (c + 1) * CHUNK)
                dtmp = spool.tile([E, CHUNK], F32, tag="dtmp")
                nc.vector.tensor_tensor(dtmp[:], ps2[c][:], wB[:, e, csl],
                                        ALU.mult)
                nc.vector.tensor_tensor(accT[:, csl], accT[:, csl], dtmp[:],
                                        ALU.add)

        def load_expert_params(e):
            w2sb = epool.tile([128, MH, E], BF16, tag="w2sb")
            nc.gpsimd.dma_start(w2sb[:], w2[e])
            b1sb = epool.tile([128, MH], F32, tag="b1sb")
            nc.gpsimd.dma_start(b1sb[:], b1[e])
            ps2 = [psB.tile([E, CHUNK], F32, tag="ps2", name=f"ps2_{e}_{c}")
                   for c in range(NCH)]
            return w2sb, b1sb, ps2

        # ---------------- expert 0 matmuls (gate hides behind them) ----
        w2sb0, b1sb0, ps2_0 = load_expert_params(0)
        emit_expert_mm(0, w2sb0, b1sb0, ps2_0)

        # ---------------- Phase C: gate ----------------
        # mu: normalize rows, transpose.
        musq = spool.tile([E, DZ], F32, tag="musq")
        mss = statp.tile([E, 1], F32, tag="mss")
        nc.scalar.activation(musq, mu_sb[:], AF.Square, accum_out=mss)
        mstd = statp.tile([E, 1], F32, tag="mstd")
        nc.scalar.activation(mstd, mss, AF.Sqrt)
        mrn = statp.tile([E, 1], F32, tag="mrn")
        nc.vector.reciprocal(mrn, mstd)
        nc.vector.tensor_scalar_mul(mun_b[:], mu_sb[:], mrn)
        for kz in range(KZ):
            tpm = psT.tile([128, E], F32, tag="tp")
            nc.tensor.matmul(tpm[:], mun_b[:, kz * 128:(kz + 1) * 128],
                             ident8b[:], start=True, stop=True)
            nc.vector.tensor_copy(munT[:, kz, :], tpm[:])

        # z: normalize rows per tile, transpose into znT.
        for bt in range(BT):
            bsl = slice(bt * 128, (bt + 1) * 128)
            zsq = spool.tile([128, DZ], F32, tag="zsq")
            zss = statp.tile([128, 1], F32, tag="zss")
            nc.scalar.activation(zsq, zall[:, bt, :], AF.Square, accum_out=zss)
            zstd = statp.tile([128, 1], F32, tag="zstd")
            nc.scalar.activation(zstd, zss, AF.Sqrt)
            zrn = statp.tile([128, 1], F32, tag="zrn")
            nc.vector.reciprocal(zrn, zstd)
            znb = spool.tile([128, DZ], BF16, tag="znb")
            nc.vector.tensor_scalar_mul(znb[:], zall[:, bt, :], zrn)
            tpz = psT.tile([128, KZ, 128], F32, tag="tp")
            for kz in range(KZ):
                nc.tensor.matmul(tpz[:, kz, :], znb[:, kz * 128:(kz + 1) * 128],
                                 identbf[:], start=True, stop=True)
            nc.vector.tensor_copy(znT[:, :, bsl], tpz[:])

        # sims + softmax per batch tile -> w_sb (f32), wT (bf16).
        for bt in range(BT):
            bsl = slice(bt * 128, (bt + 1) * 128)
            sps = psT.tile([128, E], F32, tag="tp")
            for kz in range(KZ):
                nc.tensor.matmul(sps[:], znT[:, kz, bsl], munT[:, kz, :],
                                 start=(kz == 0), stop=(kz == KZ - 1))
            ex = spool.tile([128, E], F32, tag="ex")
            if tau >= 0.25:
                # |sims/tau| <= 4: exp cannot overflow; skip max-subtract.
                nc.scalar.activation(ex[:], sps[:], AF.Exp, scale=inv_tau)
            else:
                mx = statp.tile([128, 1], F32, tag="mx")
                nc.vector.tensor_reduce(mx, sps[:], AX.X, ALU.max)
                nb = statp.tile([128, 1], F32, tag="nb")
                nc.vector.tensor_scalar_mul(nb, mx, -inv_tau)
                nc.scalar.activation(ex[:], sps[:], AF.Exp, bias=nb,
                                     scale=inv_tau)
            sm = statp.tile([128, 1], F32, tag="sm")
            nc.vector.tensor_reduce(sm, ex[:], AX.X, ALU.add)
            rsm = statp.tile([128, 1], F32, tag="rsm")
            nc.vector.reciprocal(rsm, sm)
            nc.vector.tensor_scalar_mul(w_sb[:, bt, :], ex[:], rsm)
            wbf = spool.tile([128, E], BF16, tag="wbf")
            nc.scalar.activation(wbf[:], w_sb[:, bt, :], AF.Copy)
            wtp = psT.tile([E, 128], F32, tag="tp")
            nc.tensor.matmul(wtp[:], wbf[:], identbf[:], start=True, stop=True)
            nc.vector.tensor_copy(wT[:, bsl], wtp[:])
        nc.sync.dma_start(
            w_o.rearrange("(bo bi) c -> bi bo c", bi=128), w_sb[:])

        # wB[c, e, b] = w[b, e] for all c, via selector matmuls.
        for e in range(E):
            for c in range(NCH):
                csl = slice(c * CHUNK, (c + 1) * CHUNK)
                bc = psT.tile([E, CHUNK], F32, tag="tp")
                nc.tensor.matmul(bc[:], sel[:, e * E:(e + 1) * E], wT[:, csl],
                                 start=True, stop=True)
                if e % 2 == 0:
                    nc.vector.tensor_copy(wB[:, e, csl], bc[:])
                else:
                    nc.scalar.activation(wB[:, e, csl], bc[:], AF.Copy)
        # accT init: logitsT[c, b] = sum_e b2[e, c] * w[b, e]
        for c in range(NCH):
            csl = slice(c * CHUNK, (c + 1) * CHUNK)
            bi = psT.tile([E, CHUNK], F32, tag="tp")
            nc.tensor.matmul(bi[:], b2s[:], wT[:, csl], start=True, stop=True)
            nc.vector.tensor_copy(accT[:, csl], bi[:])

        # ---------------- expert 0 drain + experts 1..7 ----------------
        emit_expert_drain(0, ps2_0)
        for e in range(1, E):
            w2sb, b1sb, ps2 = load_expert_params(e)
            emit_expert_mm(e, w2sb, b1sb, ps2)
            emit_expert_drain(e, ps2)

        # ---------------- outputs ----------------
        for bt in range(BT):
            bsl = slice(bt * 128, (bt + 1) * 128)
            ltp = psT.tile([128, E], F32, tag="tp")
            nc.tensor.matmul(ltp[:], accT[:, bsl], ident8f[:],
                             start=True, stop=True)
            nc.vector.tensor_copy(acc_out[:, bt, :], ltp[:])
        nc.sync.dma_start(
            logits_o.rearrange("(bo bi) c -> bi bo c", bi=128), acc_out[:])

    nc.compile()
    return nc


_CACHE = {}


def _prep_params(inputs):
    """Host-side: fold LN affine into W1/b1, cast+rearrange weights."""
    W1 = np.asarray(inputs["W1"], np.float32)
    b1 = np.asarray(inputs["b1"], np.float32)
    W2 = np.asarray(inputs["W2"], np.float32)
    b2 = np.asarray(inputs["b2"], np.float32)
    gam = np.asarray(inputs["ln_gamma"], np.float32)
    bet = np.asarray(inputs["ln_beta"], np.float32)
    if not np.all(gam == 1.0):
        W1 = W1 * gam[:, :, None]
    if not np.all(bet == 0.0):
        b1 = b1 + np.einsum("ed,edh->eh", bet,
                            np.asarray(inputs["W1"], np.float32))
    w1r = np.ascontiguousarray(
        W1.reshape(E, KD, 128, MH, 128).transpose(0, 3, 2, 1, 4)).astype(NPBF)
    w2r = np.ascontiguousarray(
        W2.reshape(E, MH, 128, E).transpose(0, 2, 1, 3)).astype(NPBF)
    b1r = np.ascontiguousarray(b1.reshape(E, MH, 128).transpose(0, 2, 1))
    b2r = np.ascontiguousarray(b2).astype(NPBF)
    # selector: sel[k, e*E + c] = 1 iff k == e (per-expert row-broadcast)
    selr = np.zeros((E, E * E), NPBF)
    for e in range(E):
        selr[e, e * E:(e + 1) * E] = 1.0
    return w1r, w2r, b1r, b2r, selr


def make_in_maps(inputs):
    feat = np.ascontiguousarray(np.asarray(inputs["feat"], np.float32))
    z_cat = np.ascontiguousarray(np.asarray(inputs["z_cat"], np.float32))
    mu_cat = np.ascontiguousarray(np.asarray(inputs["mu_cat"], np.float32))
    w1r, w2r, b1r, b2r, selr = _prep_params(inputs)
    in_maps = []
    for c in range(NCORES):
        rs = slice(c * BS, (c + 1) * BS)
        in_maps.append({
            "feat": feat[rs],
            "z": z_cat[rs],
            "mu": mu_cat,
            "w1": w1r,
            "w2": w2r,
            "b1": b1r,
            "b2": b2r,
            "sel": selr,
        })
    return in_maps


def kernel(**inputs):
    tau = max(1e-6, float(np.asarray(inputs["tau_gate"])))
    key = (tau,)
    if key not in _CACHE:
        _CACHE[key] = _build(tau)
    nc = _CACHE[key]

    in_maps = make_in_maps(inputs)
    res = run_bass_kernel_spmd(nc, in_maps, core_ids=list(range(NCORES)))
    outs = res.results
    logits = np.concatenate([o["logits"] for o in outs], axis=0)
    w = np.concatenate([o["w"] for o in outs], axis=0)
    return logits.astype(np.float32), w.astype(np.float32)


# revision 11
# speedup vs baseline: 1.0660x; 1.0404x over previous
"""MoE head kernel for Trainium2 (8 NeuronCores, data-parallel over batch).

Per the reference nn.Module:
  w      = softmax(cos_sim(z_cat, mu_cat) / tau)          # gate  [B, E]
  xhat   = LayerNorm(feat)                                 # affine folded into W1/b1
  h_e    = relu(xhat @ W1_e + b1_e)
  l_e    = h_e @ W2_e + b2_e
  logits = sum_e w[:, e] * l_e                             # [B, C]
returns (logits, w).

The LN affine (gamma/beta) is folded into W1/b1 on the host (exact:
x_e @ W1 = xhat @ (gamma*W1) + beta @ W1), so the device kernel has a
single shared xhat for all experts.

Sharding: batch B=16384 split 8 ways (2048 rows/core); params replicated.

Engine streams execute in emission order, so everything that is not the
expert matmul stream (LayerNorm math, xhat transposes, the whole gate)
is interleaved INTO expert 0's loop as small "slot" emissions between
matmul groups — the PE never sits behind a long serial prologue.

Per-core layout (matmul operands bf16 -> FWL weight loads, 1 cyc/row):
  - experts iterate chunk-outer (4 chunks of 512 batch rows), 16 H-tiles
    inner; mm1 accumulates hT [128, 512] over 8 K-tiles in PSUM; relu+bias
    on VectorE -> bf16; mm2 (deferred one H-tile so it never waits on
    relu) accumulates lT [8, 512] over 16 H-tiles.
  - transposes are regular matmuls against an identity (faster than PE
    transpose-mode and they count as PE-busy for the HAM clock gate).
  - gate produces w [B,E] (f32, for output), wT, and a partition-broadcast
    wB[c, e, b] = w[b, e] via tiny selector matmuls.
  - drain per (expert, chunk): logitsT += ps2 * wB[:, e, :] on VectorE
    only.  b2 is pre-accumulated into logitsT via b2.T @ wT matmuls.
  - final transposes back to [B, C] interleave into expert 7.
"""

import numpy as np
from contextlib import ExitStack

import ml_dtypes

import concourse.bass as bass
import concourse.mybir as mybir
import concourse.tile as tile
from concourse import bacc
from concourse.masks import make_identity
from concourse.bass_utils import run_bass_kernel_spmd

# Problem shapes (hardcoded per contract).
B, D, H, E, DZ = 16384, 1024, 2048, 8, 256
NCORES = 8
BS = B // NCORES            # rows per core = 2048
CHUNK = 512                 # batch chunk (PSUM bank = 512 fp32)
NCH = BS // CHUNK           # 4
BT = BS // 128              # 16 partition tiles of batch
KD = D // 128               # 8 K-tiles for mm1
MH = H // 128               # 16 M-tiles of hidden
KZ = DZ // 128              # 2 K-tiles for the gate matmul
LN_EPS = 1e-5

F32 = mybir.dt.float32
BF16 = mybir.dt.bfloat16
NPBF = ml_dtypes.bfloat16
AF = mybir.ActivationFunctionType
ALU = mybir.AluOpType
AX = mybir.AxisListType


def _build(tau: float):
    nc = bacc.Bacc(None, target_bir_lowering=False, name="moe_head")

    feat = nc.dram_tensor("feat", [BS, D], F32, kind="ExternalInput")
    z = nc.dram_tensor("z", [BS, DZ], F32, kind="ExternalInput")
    mu = nc.dram_tensor("mu", [E, DZ], F32, kind="ExternalInput")
    # w1 host layout: [e, mt, ki, ko, mi] so each strip DMA is contiguous.
    w1 = nc.dram_tensor("w1", [E, MH, 128, KD, 128], BF16, kind="ExternalInput")
    # w2 host layout: [e, ki, ko, c]
    w2 = nc.dram_tensor("w2", [E, 128, MH, E], BF16, kind="ExternalInput")
    # b1 host layout: [e, mi, mo]
    b1 = nc.dram_tensor("b1", [E, 128, MH], F32, kind="ExternalInput")
    b2 = nc.dram_tensor("b2", [E, E], BF16, kind="ExternalInput")
    sel_d = nc.dram_tensor("sel", [E, E * E], BF16, kind="ExternalInput")
    logits_o = nc.dram_tensor("logits", [BS, E], F32, kind="ExternalOutput")
    w_o = nc.dram_tensor("w", [BS, E], F32, kind="ExternalOutput")

    inv_tau = 1.0 / tau

    with tile.TileContext(nc) as tc, ExitStack() as ctx:
        persist = ctx.enter_context(tc.tile_pool(name="persist", bufs=1))
        lnpool = ctx.enter_context(tc.tile_pool(name="ln", bufs=3))
        xhpool = ctx.enter_context(tc.tile_pool(name="xh", bufs=6))
        statp = ctx.enter_context(tc.tile_pool(name="stat", bufs=4))
        wpool = ctx.enter_context(tc.tile_pool(name="w1s", bufs=MH))
        epool = ctx.enter_context(tc.tile_pool(name="eparam", bufs=2))
        hpool = ctx.enter_context(tc.tile_pool(name="h", bufs=4))
        spool = ctx.enter_context(tc.tile_pool(name="small", bufs=3))
        psA = ctx.enter_context(tc.tile_pool(name="psA", bufs=2, space="PSUM"))
        psB = ctx.enter_context(tc.tile_pool(name="psB", bufs=2, space="PSUM"))
        psT = ctx.enter_context(tc.tile_pool(name="psT", bufs=4, space="PSUM"))

        # ---- persistent SBUF ----
        xhatT_c = [persist.tile([128, KD, CHUNK], BF16, name=f"xhatT{c}")
                   for c in range(NCH)]
        znT = persist.tile([128, KZ, BS], BF16)
        munT = persist.tile([128, KZ, E], BF16)
        wT = persist.tile([E, BS], BF16)          # gate weights, transposed
        wB = persist.tile([E, E, BS], BF16)       # w[b, e] bcast to C partitions
        w_sb = persist.tile([128, BT, E], F32)    # gate weights [B, E]
        accT = persist.tile([E, BS], F32)         # logitsT accumulator
        acc_out = persist.tile([128, BT, E], F32)
        identbf = persist.tile([128, 128], BF16)
        ident8b = persist.tile([E, E], BF16)
        ident8f = persist.tile([E, E], F32)
        sel = persist.tile([E, E * E], BF16)
        b2s = persist.tile([E, E], BF16)
        zall = persist.tile([128, BT, DZ], F32)
        mu_sb = persist.tile([E, DZ], F32)
        mun_b = persist.tile([E, DZ], BF16)
        eps_sb = persist.tile([128, 1], F32)

        make_identity(nc, identbf)
        make_identity(nc, ident8b)
        make_identity(nc, ident8f)
        nc.vector.memset(eps_sb[:], LN_EPS)

        # activations on sync queue; gate inputs on scalar queue;
        # weights on gpsimd queue (independent DMA streams).
        nc.scalar.dma_start(mu_sb[:], mu[:, :])
        nc.scalar.dma_start(zall[:], z.rearrange("(bo bi) d -> bi bo d", bi=128))
        nc.gpsimd.dma_start(b2s[:], b2[:, :])
        nc.gpsimd.dma_start(sel[:], sel_d[:, :])

        xh_tiles = [None] * BT

        def emit_ln(bt):
            """LayerNorm math for one batch tile (DVE/ScalarE only)."""
            bsl = slice(bt * 128, (bt + 1) * 128)
            ft = lnpool.tile([128, D], F32, tag="ft")
            nc.sync.dma_start(ft[:], feat[bsl, :])
            s1 = statp.tile([128, 1], F32, tag="s1")
            nc.vector.tensor_reduce(s1, ft[:], AX.X, ALU.add)
            sq = lnpool.tile([128, D], F32, tag="sq")
            ss = statp.tile([128, 1], F32, tag="ss")
            nc.scalar.activation(sq, ft[:], AF.Square, accum_out=ss)
            nm = statp.tile([128, 1], F32, tag="nm")
            nc.vector.tensor_scalar_mul(nm, s1, -1.0 / D)
            ms = statp.tile([128, 1], F32, tag="ms")
            nc.vector.tensor_tensor(ms, nm, nm, ALU.mult)
            vv = statp.tile([128, 1], F32, tag="vv")
            nc.vector.scalar_tensor_tensor(vv, ss, 1.0 / D, ms, ALU.mult,
                                           ALU.subtract)
            std = statp.tile([128, 1], F32, tag="std")
            nc.scalar.activation(std, vv, AF.Sqrt, bias=eps_sb[:])
            rs = statp.tile([128, 1], F32, tag="rs")
            nc.vector.reciprocal(rs, std)
            xh = xhpool.tile([128, D], BF16, tag="xh", name=f"xh_{bt}")
            nc.vector.tensor_scalar(xh[:], ft[:], nm, rs, ALU.add, ALU.mult)
            xh_tiles[bt] = xh

        def emit_xhat_transpose(c):
            """Transpose this chunk's 4 LN'd tiles into xhatT_c[c]."""
            for j in range(4):
                bt = 4 * c + j
                xh = xh_tiles[bt]
                lo = j * 128
                for g in range(2):
                    tp = psT.tile([128, 4, 128], F32, tag="tp")
                    for jj in range(4):
                        kd = g * 4 + jj
                        nc.tensor.matmul(
                            tp[:, jj, :], xh[:, kd * 128:(kd + 1) * 128],
                            identbf[:], start=True, stop=True)
                    dst = xhatT_c[c][:, g * 4:(g + 1) * 4, lo:lo + 128]
                    if (bt + g) % 2 == 0:
                        nc.scalar.activation(dst, tp[:], AF.Copy)
                    else:
                        nc.vector.tensor_copy(dst, tp[:])

        def emit_mu_norm():
            musq = spool.tile([E, DZ], F32, tag="musq")
            mss = statp.tile([E, 1], F32, tag="mss")
            nc.scalar.activation(musq, mu_sb[:], AF.Square, accum_out=mss)
            mstd = statp.tile([E, 1], F32, tag="mstd")
            nc.scalar.activation(mstd, mss, AF.Sqrt)
            mrn = statp.tile([E, 1], F32, tag="mrn")
            nc.vector.reciprocal(mrn, mstd)
            nc.vector.tensor_scalar_mul(mun_b[:], mu_sb[:], mrn)

        def emit_mu_transpose():
            for kz in range(KZ):
                tpm = psT.tile([128, E], F32, tag="tp")
                nc.tensor.matmul(tpm[:], mun_b[:, kz * 128:(kz + 1) * 128],
                                 ident8b[:], start=True, stop=True)
                nc.vector.tensor_copy(munT[:, kz, :], tpm[:])

        def emit_z(bt):
            """Normalize z rows for one tile + transpose into znT."""
            bsl = slice(bt * 128, (bt + 1) * 128)
            zsq = spool.tile([128, DZ], F32, tag="zsq")
            zss = statp.tile([128, 1], F32, tag="zss")
            nc.scalar.activation(zsq, zall[:, bt, :], AF.Square, accum_out=zss)
            zstd = statp.tile([128, 1], F32, tag="zstd")
            nc.scalar.activation(zstd, zss, AF.Sqrt)
            zrn = statp.tile([128, 1], F32, tag="zrn")
            nc.vector.reciprocal(zrn, zstd)
            znb = spool.tile([128, DZ], BF16, tag="znb")
            nc.vector.tensor_scalar_mul(znb[:], zall[:, bt, :], zrn)
            tpz = psT.tile([128, KZ, 128], F32, tag="tp")
            for kz in range(KZ):
                nc.tensor.matmul(tpz[:, kz, :], znb[:, kz * 128:(kz + 1) * 128],
                                 identbf[:], start=True, stop=True)
            nc.vector.tensor_copy(znT[:, :, bsl], tpz[:])

        def emit_sims(bt):
            """cos-sims + softmax for one tile -> w_sb row block + wT."""
            bsl = slice(bt * 128, (bt + 1) * 128)
            sps = psT.tile([128, E], F32, tag="tp")
            for kz in range(KZ):
                nc.tensor.matmul(sps[:], znT[:, kz, bsl], munT[:, kz, :],
                                 start=(kz == 0), stop=(kz == KZ - 1))
            ex = spool.tile([128, E], F32, tag="ex")
            if tau >= 0.25:
                # |sims/tau| <= 4: exp cannot overflow; skip max-subtract.
                nc.scalar.activation(ex[:], sps[:], AF.Exp, scale=inv_tau)
            else:
                mx = statp.tile([128, 1], F32, tag="mx")
                nc.vector.tensor_reduce(mx, sps[:], AX.X, ALU.max)
                nb = statp.tile([128, 1], F32, tag="nb")
                nc.vector.tensor_scalar_mul(nb, mx, -inv_tau)
                nc.scalar.activation(ex[:], sps[:], AF.Exp, bias=nb,
                                     scale=inv_tau)
            sm = statp.tile([128, 1], F32, tag="sm")
            nc.vector.tensor_reduce(sm, ex[:], AX.X, ALU.add)
            rsm = statp.tile([128, 1], F32, tag="rsm")
            nc.vector.reciprocal(rsm, sm)
            nc.vector.tensor_scalar_mul(w_sb[:, bt, :], ex[:], rsm)
            wbf = spool.tile([128, E], BF16, tag="wbf")
            nc.scalar.activation(wbf[:], w_sb[:, bt, :], AF.Copy)
            wtp = psT.tile([E, 128], F32, tag="tp")
            nc.tensor.matmul(wtp[:], wbf[:], identbf[:], start=True, stop=True)
            nc.vector.tensor_copy(wT[:, bsl], wtp[:])

        def emit_wb(ch):
            """wB[c, e, b] = w[b, e] for this chunk + accT init with b2."""
            csl = slice(ch * CHUNK, (ch + 1) * CHUNK)
            for e in range(E):
                bc = psT.tile([E, CHUNK], F32, tag="tp")
                nc.tensor.matmul(bc[:], sel[:, e * E:(e + 1) * E], wT[:, csl],
                                 start=True, stop=True)
                if e % 2 == 0:
                    nc.vector.tensor_copy(wB[:, e, csl], bc[:])
                else:
                    nc.scalar.activation(wB[:, e, csl], bc[:], AF.Copy)
            bi = psT.tile([E, CHUNK], F32, tag="tp")
            nc.tensor.matmul(bi[:], b2s[:], wT[:, csl], start=True, stop=True)
            nc.vector.tensor_copy(accT[:, csl], bi[:])

        def slot_cb(c, mt):
            """Gate/LN work interleaved into expert 0's PE stream."""
            if mt in (2, 5, 8, 11) and c < NCH - 1:
                emit_ln(4 * (c + 1) + (mt - 2) // 3)
            if mt in (1, 4, 7, 10):
                if c == 0 and mt == 1:
                    emit_mu_transpose()
                emit_z(4 * c + (mt - 1) // 3)
            if mt in (3, 6, 9, 12):
                emit_sims(4 * c + (mt - 3) // 3)
            if mt == 14:
                emit_wb(c)
                if c == NCH - 1:
                    nc.sync.dma_start(
                        w_o.rearrange("(bo bi) c -> bi bo c", bi=128), w_sb[:])

        def emit_expert(e):
            w2sb = epool.tile([128, MH, E], BF16, tag="w2sb")
            nc.gpsimd.dma_start(w2sb[:], w2[e])
            b1sb = epool.tile([128, MH], F32, tag="b1sb")
            nc.gpsimd.dma_start(b1sb[:], b1[e])
            strips = [None] * MH
            for c in range(NCH):
                if e == 0:
                    emit_xhat_transpose(c)
                ps2 = psB.tile([E, CHUNK], F32, tag="ps2", name=f"ps2_{e}_{c}")
                h_prev = None
                for mt in range(MH):
                    if e == 0:
                        slot_cb(c, mt)
                    if c == 0:
                        strips[mt] = wpool.tile([128, KD, 128], BF16,
                                                tag="w1s", name=f"w1s_{e}_{mt}")
                        nc.gpsimd.dma_start(strips[mt][:], w1[e, mt])
                    ps1 = psA.tile([128, CHUNK], F32, tag="ps1")
                    for k in range(KD):
                        nc.tensor.matmul(
                            ps1[:], strips[mt][:, k, :], xhatT_c[c][:, k, :],
                            start=(k == 0), stop=(k == KD - 1))
                    h = hpool.tile([128, CHUNK], BF16, tag="h")
                    nc.vector.tensor_scalar(h[:], ps1[:], b1sb[:, mt:mt + 1],
                                            0.0, ALU.add, ALU.max)
                    if mt >= 1:
                        nc.tensor.matmul(ps2[:], w2sb[:, mt - 1, :], h_prev[:],
                                         start=(mt == 1), stop=False)
                    h_prev = h
                nc.tensor.matmul(ps2[:], w2sb[:, MH - 1, :], h_prev[:],
                                 start=False, stop=True)
                # drain chunk: logitsT += ps2 * w[:, e] (broadcast layout)
                csl = slice(c * CHUNK, (c + 1) * CHUNK)
                dtmp = spool.tile([E, CHUNK], F32, tag="dtmp")
                nc.vector.tensor_tensor(dtmp[:], ps2[:], wB[:, e, csl],
                                        ALU.mult)
                nc.vector.tensor_tensor(accT[:, csl], accT[:, csl], dtmp[:],
                                        ALU.add)
                if e == E - 1:
                    for j in range(4):
                        bt = 4 * c + j
                        bsl = slice(bt * 128, (bt + 1) * 128)
                        ltp = psT.tile([128, E], F32, tag="tp")
                        nc.tensor.matmul(ltp[:], accT[:, bsl], ident8f[:],
                                         start=True, stop=True)
                        nc.vector.tensor_copy(acc_out[:, bt, :], ltp[:])

        # prologue: LN for chunk 0's tiles + mu normalization
        for bt in range(4):
            emit_ln(bt)
        emit_mu_norm()

        for e in range(E):
            emit_expert(e)

        nc.sync.dma_start(
            logits_o.rearrange("(bo bi) c -> bi bo c", bi=128), acc_out[:])

    nc.compile()
    return nc


_CACHE = {}


def _prep_params(inputs):
    """Host-side: fold LN affine into W1/b1, cast+rearrange weights."""
    W1 = np.asarray(inputs["W1"], np.float32)
    b1 = np.asarray(inputs["b1"], np.float32)
    W2 = np.asarray(inputs["W2"], np.float32)
    b2 = np.asarray(inputs["b2"], np.float32)
    gam = np.asarray(inputs["ln_gamma"], np.float32)
    bet = np.asarray(inputs["ln_beta"], np.float32)
    if not np.all(gam == 1.0):
        W1 = W1 * gam[:, :, None]
    if not np.all(bet == 0.0):
        b1 = b1 + np.einsum("ed,edh->eh", bet,
                            np.asarray(inputs["W1"], np.float32))
    w1r = np.ascontiguousarray(
        W1.reshape(E, KD, 128, MH, 128).transpose(0, 3, 2, 1, 4)).astype(NPBF)
    w2r = np.ascontiguousarray(
        W2.reshape(E, MH, 128, E).transpose(0, 2, 1, 3)).astype(NPBF)
    b1r = np.ascontiguousarray(b1.reshape(E, MH, 128).transpose(0, 2, 1))
    b2r = np.ascontiguousarray(b2).astype(NPBF)
    # selector: sel[k, e*E + c] = 1 iff k == e (per-expert row-broadcast)
    selr = np.zeros((E, E * E), NPBF)
    for e in range(E):
        selr[e, e * E:(e + 1) * E] = 1.0
    return w1r, w2r, b1r, b2r, selr


def make_in_maps(inputs):
    feat = np.ascontiguousarray(np.asarray(inputs["feat"], np.float32))
    z_cat = np.ascontiguousarray(np.asarray(inputs["z_cat"], np.float32))
    mu_cat = np.ascontiguousarray(np.asarray(inputs["mu_cat"], np.float32))
    w1r, w2r, b1r, b2r, selr = _prep_params(inputs)
    in_maps = []
    for c in range(NCORES):
        rs = slice(c * BS, (c + 1) * BS)
        in_maps.append({
            "feat": feat[rs],
            "z": z_cat[rs],
            "mu": mu_cat,
            "w1": w1r,
            "w2": w2r,
            "b1": b1r,
            "b2": b2r,
            "sel": selr,
        })
    return in_maps


def kernel(**inputs):
    tau = max(1e-6, float(np.asarray(inputs["tau_gate"])))
    key = (tau,)
    if key not in _CACHE:
        _CACHE[key] = _build(tau)
    nc = _CACHE[key]

    in_maps = make_in_maps(inputs)
    res = run_bass_kernel_spmd(nc, in_maps, core_ids=list(range(NCORES)))
    outs = res.results
    logits = np.concatenate([o["logits"] for o in outs], axis=0)
    w = np.concatenate([o["w"] for o in outs], axis=0)
    return logits.astype(np.float32), w.astype(np.float32)


# revision 12
# speedup vs baseline: 1.1772x; 1.1043x over previous
"""MoE head kernel for Trainium2 (8 NeuronCores, data-parallel over batch).

Per the reference nn.Module:
  w      = softmax(cos_sim(z_cat, mu_cat) / tau)          # gate  [B, E]
  xhat   = LayerNorm(feat)                                 # affine folded into W1/b1
  h_e    = relu(xhat @ W1_e + b1_e)
  l_e    = h_e @ W2_e + b2_e
  logits = sum_e w[:, e] * l_e                             # [B, C]
returns (logits, w).

The LN affine (gamma/beta) is folded into W1/b1 on the host (exact:
x_e @ W1 = xhat @ (gamma*W1) + beta @ W1), so the device kernel has a
single shared xhat for all experts.

Sharding: batch B=16384 split 8 ways (2048 rows/core); params replicated.

Engine streams execute in emission order, so everything that is not the
expert matmul stream (LayerNorm math, xhat transposes, the whole gate)
is interleaved INTO expert 0's loop as small "slot" emissions between
matmul groups — the PE never sits behind a long serial prologue.

Per-core layout (matmul operands bf16 -> FWL weight loads, 1 cyc/row):
  - experts iterate chunk-outer (4 chunks of 512 batch rows), 16 H-tiles
    inner; mm1 accumulates hT [128, 512] over 8 K-tiles in PSUM; relu+bias
    on ScalarE into a persistent h buffer [128, 16, 512] bf16.
  - mm2 runs as a 16-matmul BURST per chunk into one PSUM bank (weight
    loads pipeline within the burst), deferred into the next chunk's
    stream so it never waits on relu.
  - transposes are regular matmuls against an identity (faster than PE
    transpose-mode and they count as PE-busy for the HAM clock gate).
  - gate produces w [B,E] (f32, for output), wT, and a partition-broadcast
    wB[c, e, b] = w[b, e] via tiny selector matmuls.
  - drain per (expert, chunk): logitsT += ps2 * wB[:, e, :] on VectorE
    only.  b2 is pre-accumulated into logitsT via b2.T @ wT matmuls.
  - final transposes back to [B, C] interleave into the tail.
"""

import numpy as np
from contextlib import ExitStack

import ml_dtypes

import concourse.bass as bass
import concourse.mybir as mybir
import concourse.tile as tile
from concourse import bacc
from concourse.masks import make_identity
from concourse.bass_utils import run_bass_kernel_spmd

# Problem shapes (hardcoded per contract).
B, D, H, E, DZ = 16384, 1024, 2048, 8, 256
NCORES = 8
BS = B // NCORES            # rows per core = 2048
CHUNK = 512                 # batch chunk (PSUM bank = 512 fp32)
NCH = BS // CHUNK           # 4
BT = BS // 128              # 16 partition tiles of batch
KD = D // 128               # 8 K-tiles for mm1
MH = H // 128               # 16 M-tiles of hidden
KZ = DZ // 128              # 2 K-tiles for the gate matmul
LN_EPS = 1e-5

F32 = mybir.dt.float32
BF16 = mybir.dt.bfloat16
NPBF = ml_dtypes.bfloat16
AF = mybir.ActivationFunctionType
ALU = mybir.AluOpType
AX = mybir.AxisListType


def _build(tau: float):
    nc = bacc.Bacc(None, target_bir_lowering=False, name="moe_head")

    feat = nc.dram_tensor("feat", [BS, D], F32, kind="ExternalInput")
    z = nc.dram_tensor("z", [BS, DZ], F32, kind="ExternalInput")
    mu = nc.dram_tensor("mu", [E, DZ], F32, kind="ExternalInput")
    # w1 host layout: [e, mt, ki, ko, mi] so each strip DMA is contiguous.
    w1 = nc.dram_tensor("w1", [E, MH, 128, KD, 128], BF16, kind="ExternalInput")
    # w2 host layout: [e, ki, ko, c]
    w2 = nc.dram_tensor("w2", [E, 128, MH, E], BF16, kind="ExternalInput")
    # b1 host layout: [e, mi, mo]
    b1 = nc.dram_tensor("b1", [E, 128, MH], F32, kind="ExternalInput")
    b2 = nc.dram_tensor("b2", [E, E], BF16, kind="ExternalInput")
    sel_d = nc.dram_tensor("sel", [E, E * E], BF16, kind="ExternalInput")
    logits_o = nc.dram_tensor("logits", [BS, E], F32, kind="ExternalOutput")
    w_o = nc.dram_tensor("w", [BS, E], F32, kind="ExternalOutput")

    inv_tau = 1.0 / tau

    with tile.TileContext(nc) as tc, ExitStack() as ctx:
        persist = ctx.enter_context(tc.tile_pool(name="persist", bufs=1))
        ftpool = ctx.enter_context(tc.tile_pool(name="ftp", bufs=5))
        sqpool = ctx.enter_context(tc.tile_pool(name="sqp", bufs=1))
        xhpool = ctx.enter_context(tc.tile_pool(name="xh", bufs=6))
        statp = ctx.enter_context(tc.tile_pool(name="stat", bufs=4))
        wpool = ctx.enter_context(tc.tile_pool(name="w1s", bufs=MH))
        epool = ctx.enter_context(tc.tile_pool(name="eparam", bufs=2))
        spool = ctx.enter_context(tc.tile_pool(name="small", bufs=3))
        psA = ctx.enter_context(tc.tile_pool(name="psA", bufs=2, space="PSUM"))
        psB = ctx.enter_context(tc.tile_pool(name="psB", bufs=2, space="PSUM"))
        psT = ctx.enter_context(tc.tile_pool(name="psT", bufs=4, space="PSUM"))

        # ---- persistent SBUF ----
        xhatT_c = [persist.tile([128, KD, CHUNK], BF16, name=f"xhatT{c}")
                   for c in range(NCH)]
        hall = [persist.tile([128, MH, CHUNK], BF16, name=f"hall{p}")
                for p in range(2)]
        znT = persist.tile([128, KZ, BS], BF16)
        munT = persist.tile([128, KZ, E], BF16)
        wT = persist.tile([E, BS], BF16)          # gate weights, transposed
        wB = persist.tile([E, E, BS], BF16)       # w[b, e] bcast to C partitions
        w_sb = persist.tile([128, BT, E], F32)    # gate weights [B, E]
        accT = persist.tile([E, BS], F32)         # logitsT accumulator
        acc_out = persist.tile([128, BT, E], F32)
        identbf = persist.tile([128, 128], BF16)
        ident8b = persist.tile([E, E], BF16)
        ident8f = persist.tile([E, E], F32)
        sel = persist.tile([E, E * E], BF16)
        b2s = persist.tile([E, E], BF16)
        mu_sb = persist.tile([E, DZ], F32)
        mun_b = persist.tile([E, DZ], BF16)
        eps_sb = persist.tile([128, 1], F32)

        make_identity(nc, identbf)
        make_identity(nc, ident8b)
        make_identity(nc, ident8f)
        nc.vector.memset(eps_sb[:], LN_EPS)

        # activations on sync queue; gate inputs on scalar queue;
        # weights on gpsimd queue (independent DMA streams).
        nc.scalar.dma_start(mu_sb[:], mu[:, :])
        nc.gpsimd.dma_start(b2s[:], b2[:, :])
        nc.gpsimd.dma_start(sel[:], sel_d[:, :])

        xh_tiles = [None] * BT
        ln_stats = [None] * BT

        def emit_ln_a(bt):
            """LN part A: load + the two row reductions (ScalarE)."""
            bsl = slice(bt * 128, (bt + 1) * 128)
            ft = ftpool.tile([128, D], F32, tag="ft", name=f"ft_{bt}")
            nc.sync.dma_start(ft[:], feat[bsl, :])
            junk = sqpool.tile([128, D], F32, tag="sq", name=f"junk_{bt}")
            s1 = statp.tile([128, 1], F32, tag="s1", name=f"s1_{bt}")
            nc.scalar.activation(junk, ft[:], AF.Copy, accum_out=s1)
            ss = statp.tile([128, 1], F32, tag="ss", name=f"ss_{bt}")
            nc.scalar.activation(junk, ft[:], AF.Square, accum_out=ss)
            ln_stats[bt] = (ft, s1, ss)

        def emit_ln_b(bt):
            """LN part B: stats -> xhat (DVE + one scalar Sqrt)."""
            ft, s1, ss = ln_stats[bt]
            nm = statp.tile([128, 1], F32, tag="nm", name=f"nm_{bt}")
            nc.vector.tensor_scalar_mul(nm, s1, -1.0 / D)
            ms = statp.tile([128, 1], F32, tag="ms", name=f"ms_{bt}")
            nc.vector.tensor_tensor(ms, nm, nm, ALU.mult)
            vv = statp.tile([128, 1], F32, tag="vv", name=f"vv_{bt}")
            nc.vector.scalar_tensor_tensor(vv, ss, 1.0 / D, ms, ALU.mult,
                                           ALU.subtract)
            std = statp.tile([128, 1], F32, tag="std", name=f"std_{bt}")
            nc.scalar.activation(std, vv, AF.Sqrt, bias=eps_sb[:])
            rs = statp.tile([128, 1], F32, tag="rs", name=f"rs_{bt}")
            nc.vector.reciprocal(rs, std)
            xh = xhpool.tile([128, D], BF16, tag="xh", name=f"xh_{bt}")
            nc.vector.tensor_scalar(xh[:], ft[:], nm, rs, ALU.add, ALU.mult)
            xh_tiles[bt] = xh

        def emit_xhat_transpose(c):
            """Transpose this chunk's 4 LN'd tiles into xhatT_c[c]."""
            for j in range(4):
                bt = 4 * c + j
                xh = xh_tiles[bt]
                lo = j * 128
                for g in range(2):
                    tp = psT.tile([128, 4, 128], F32, tag="tp")
                    for jj in range(4):
                        kd = g * 4 + jj
                        nc.tensor.matmul(
                            tp[:, jj, :], xh[:, kd * 128:(kd + 1) * 128],
                            identbf[:], start=True, stop=True)
                    dst = xhatT_c[c][:, g * 4:(g + 1) * 4, lo:lo + 128]
                    if (bt + g) % 2 == 0:
                        nc.scalar.activation(dst, tp[:], AF.Copy)
                    else:
                        nc.vector.tensor_copy(dst, tp[:])

        def emit_mu_norm():
            musq = spool.tile([E, DZ], F32, tag="musq")
            mss = statp.tile([E, 1], F32, tag="mss")
            nc.scalar.activation(musq, mu_sb[:], AF.Square, accum_out=mss)
            mstd = statp.tile([E, 1], F32, tag="mstd")
            nc.scalar.activation(mstd, mss, AF.Sqrt)
            mrn = statp.tile([E, 1], F32, tag="mrn")
            nc.vector.reciprocal(mrn, mstd)
            nc.vector.tensor_scalar_mul(mun_b[:], mu_sb[:], mrn)

        def emit_mu_transpose():
            for kz in range(KZ):
                tpm = psT.tile([128, E], F32, tag="tp")
                nc.tensor.matmul(tpm[:], mun_b[:, kz * 128:(kz + 1) * 128],
                                 ident8b[:], start=True, stop=True)
                nc.vector.tensor_copy(munT[:, kz, :], tpm[:])

        def emit_z(bt):
            """Normalize z rows for one tile + transpose into znT."""
            bsl = slice(bt * 128, (bt + 1) * 128)
            zt = spool.tile([128, DZ], F32, tag="zt", name=f"zt_{bt}")
            nc.scalar.dma_start(zt[:], z[bsl, :])
            zsq = spool.tile([128, DZ], F32, tag="zsq")
            zss = statp.tile([128, 1], F32, tag="zss")
            nc.scalar.activation(zsq, zt[:], AF.Square, accum_out=zss)
            zstd = statp.tile([128, 1], F32, tag="zstd")
            nc.scalar.activation(zstd, zss, AF.Sqrt)
            zrn = statp.tile([128, 1], F32, tag="zrn")
            nc.vector.reciprocal(zrn, zstd)
            znb = spool.tile([128, DZ], BF16, tag="znb")
            nc.vector.tensor_scalar_mul(znb[:], zt[:], zrn)
            tpz = psT.tile([128, KZ, 128], F32, tag="tp")
            for kz in range(KZ):
                nc.tensor.matmul(tpz[:, kz, :], znb[:, kz * 128:(kz + 1) * 128],
                                 identbf[:], start=True, stop=True)
            nc.vector.tensor_copy(znT[:, :, bsl], tpz[:])

        def emit_sims(bt):
            """cos-sims + softmax for one tile -> w_sb row block + wT."""
            bsl = slice(bt * 128, (bt + 1) * 128)
            sps = psT.tile([128, E], F32, tag="tp")
            for kz in range(KZ):
                nc.tensor.matmul(sps[:], znT[:, kz, bsl], munT[:, kz, :],
                                 start=(kz == 0), stop=(kz == KZ - 1))
            ex = spool.tile([128, E], F32, tag="ex")
            if tau >= 0.25:
                # |sims/tau| <= 4: exp cannot overflow; skip max-subtract.
                nc.scalar.activation(ex[:], sps[:], AF.Exp, scale=inv_tau)
            else:
                mx = statp.tile([128, 1], F32, tag="mx")
                nc.vector.tensor_reduce(mx, sps[:], AX.X, ALU.max)
                nb = statp.tile([128, 1], F32, tag="nb")
                nc.vector.tensor_scalar_mul(nb, mx, -inv_tau)
                nc.scalar.activation(ex[:], sps[:], AF.Exp, bias=nb,
                                     scale=inv_tau)
            sm = statp.tile([128, 1], F32, tag="sm")
            nc.vector.tensor_reduce(sm, ex[:], AX.X, ALU.add)
            rsm = statp.tile([128, 1], F32, tag="rsm")
            nc.vector.reciprocal(rsm, sm)
            nc.vector.tensor_scalar_mul(w_sb[:, bt, :], ex[:], rsm)
            wbf = spool.tile([128, E], BF16, tag="wbf")
            nc.vector.tensor_scalar_mul(wbf[:], ex[:], rsm)
            wtp = psT.tile([E, 128], F32, tag="tp")
            nc.tensor.matmul(wtp[:], wbf[:], identbf[:], start=True, stop=True)
            nc.vector.tensor_copy(wT[:, bsl], wtp[:])

        def emit_wb(ch):
            """wB[c, e, b] = w[b, e] for this chunk + accT init with b2."""
            csl = slice(ch * CHUNK, (ch + 1) * CHUNK)
            for e in range(E):
                bc = psT.tile([E, CHUNK], F32, tag="tp")
                nc.tensor.matmul(bc[:], sel[:, e * E:(e + 1) * E], wT[:, csl],
                                 start=True, stop=True)
                if e % 2 == 0:
                    nc.vector.tensor_copy(wB[:, e, csl], bc[:])
                else:
                    nc.scalar.activation(wB[:, e, csl], bc[:], AF.Copy)
            bi = psT.tile([E, CHUNK], F32, tag="tp")
            nc.tensor.matmul(bi[:], b2s[:], wT[:, csl], start=True, stop=True)
            nc.vector.tensor_copy(accT[:, csl], bi[:])

        def slot_cb(c, mt):
            """Gate/LN work interleaved into expert 0's PE stream."""
            if mt in (1, 4, 7, 10):
                if c == 0 and mt == 1:
                    emit_mu_transpose()
                emit_z(4 * c + (mt - 1) // 3)
            if mt in (2, 5, 8, 11) and c < NCH - 1:
                emit_ln_a(4 * (c + 1) + (mt - 2) // 3)
            if mt in (3, 6, 9, 12):
                emit_sims(4 * c + (mt - 3) // 3)
            if mt in (4, 7, 10, 13) and c < NCH - 1:
                emit_ln_b(4 * (c + 1) + (mt - 4) // 3)
            if mt == 14:
                emit_wb(c)
                if c == NCH - 1:
                    nc.sync.dma_start(
                        w_o.rearrange("(bo bi) c -> bi bo c", bi=128), w_sb[:])

        # pending mm2 burst/drain state, flushed inside the next chunk
        pending = []

        def flush_pending():
            if not pending:
                return
            e, c, ps2, hbuf, w2sb = pending.pop()
            for mt in range(MH):
                nc.tensor.matmul(ps2[:], w2sb[:, mt, :], hbuf[:, mt, :],
                                 start=(mt == 0), stop=(mt == MH - 1))
            csl = slice(c * CHUNK, (c + 1) * CHUNK)
            dtmp = spool.tile([E, CHUNK], F32, tag="dtmp")
            nc.vector.tensor_tensor(dtmp[:], ps2[:], wB[:, e, csl], ALU.mult)
            nc.vector.tensor_tensor(accT[:, csl], accT[:, csl], dtmp[:],
                                    ALU.add)
            if e == E - 1:
                for j in range(4):
                    bt = 4 * c + j
                    bsl = slice(bt * 128, (bt + 1) * 128)
                    ltp = psT.tile([128, E], F32, tag="tp")
                    nc.tensor.matmul(ltp[:], accT[:, bsl], ident8f[:],
                                     start=True, stop=True)
                    nc.vector.tensor_copy(acc_out[:, bt, :], ltp[:])

        def emit_expert(e):
            w2sb = epool.tile([128, MH, E], BF16, tag="w2sb",
                              name=f"w2sb_{e}")
            nc.gpsimd.dma_start(w2sb[:], w2[e])
            b1sb = epool.tile([128, MH], F32, tag="b1sb", name=f"b1sb_{e}")
            nc.gpsimd.dma_start(b1sb[:], b1[e])
            strips = [None] * MH
            for c in range(NCH):
                if e == 0:
                    emit_xhat_transpose(c)
                ps2 = psB.tile([E, CHUNK], F32, tag="ps2", name=f"ps2_{e}_{c}")
                hbuf = hall[(e * NCH + c) % 2]
                for mt in range(MH):
                    if e == 0:
                        slot_cb(c, mt)
                    if c == 0:
                        strips[mt] = wpool.tile([128, KD, 128], BF16,
                                                tag="w1s", name=f"w1s_{e}_{mt}")
                        nc.gpsimd.dma_start(strips[mt][:], w1[e, mt])
                    ps1 = psA.tile([128, CHUNK], F32, tag="ps1")
                    for k in range(KD):
                        nc.tensor.matmul(
                            ps1[:], strips[mt][:, k, :], xhatT_c[c][:, k, :],
                            start=(k == 0), stop=(k == KD - 1))
                    nc.scalar.activation(hbuf[:, mt, :], ps1[:], AF.Relu,
                                         bias=b1sb[:, mt:mt + 1])
                    if mt == 0:
                        flush_pending()
                pending.append((e, c, ps2, hbuf, w2sb))

        # prologue: LN for chunk 0's tiles + mu normalization
        emit_ln_a(0)
        emit_ln_a(1)
        emit_ln_b(0)
        emit_ln_a(2)
        emit_ln_b(1)
        emit_ln_a(3)
        emit_ln_b(2)
        emit_ln_b(3)
        emit_mu_norm()

        for e in range(E):
            emit_expert(e)
        flush_pending()

        nc.sync.dma_start(
            logits_o.rearrange("(bo bi) c -> bi bo c", bi=128), acc_out[:])

    nc.compile()
    return nc


_CACHE = {}


def _prep_params(inputs):
    """Host-side: fold LN affine into W1/b1, cast+rearrange weights."""
    W1 = np.asarray(inputs["W1"], np.float32)
    b1 = np.asarray(inputs["b1"], np.float32)
    W2 = np.asarray(inputs["W2"], np.float32)
    b2 = np.asarray(inputs["b2"], np.float32)
    gam = np.asarray(inputs["ln_gamma"], np.float32)
    bet = np.asarray(inputs["ln_beta"], np.float32)
    if not np.all(gam == 1.0):
        W1 = W1 * gam[:, :, None]
    if not np.all(bet == 0.0):
        b1 = b1 + np.einsum("ed,edh->eh", bet,
                            np.asarray(inputs["W1"], np.float32))
    w1r = np.ascontiguousarray(
        W1.reshape(E, KD, 128, MH, 128).transpose(0, 3, 2, 1, 4)).astype(NPBF)
    w2r = np.ascontiguousarray(
        W2.reshape(E, MH, 128, E).transpose(0, 2, 1, 3)).astype(NPBF)
    b1r = np.ascontiguousarray(b1.reshape(E, MH, 128).transpose(0, 2, 1))
    b2r = np.ascontiguousarray(b2).astype(NPBF)
    # selector: sel[k, e*E + c] = 1 iff k == e (per-expert row-broadcast)
    selr = np.zeros((E, E * E), NPBF)
    for e in range(E):
        selr[e, e * E:(e + 1) * E] = 1.0
    return w1r, w2r, b1r, b2r, selr


def make_in_maps(inputs):
    feat = np.ascontiguousarray(np.asarray(inputs["feat"], np.float32))
    z_cat = np.ascontiguousarray(np.asarray(inputs["z_cat"], np.float32))
    mu_cat = np.ascontiguousarray(np.asarray(inputs["mu_cat"], np.float32))
    w1r, w2r, b1r, b2r, selr = _prep_params(inputs)
    in_maps = []
    for c in range(NCORES):
        rs = slice(c * BS, (c + 1) * BS)
        in_maps.append({
            "feat": feat[rs],
            "z": z_cat[rs],
            "mu": mu_cat,
            "w1": w1r,
            "w2": w2r,
            "b1": b1r,
            "b2": b2r,
            "sel": selr,
        })
    return in_maps


def kernel(**inputs):
    tau = max(1e-6, float(np.asarray(inputs["tau_gate"])))
    key = (tau,)
    if key not in _CACHE:
        _CACHE[key] = _build(tau)
    nc = _CACHE[key]

    in_maps = make_in_maps(inputs)
    res = run_bass_kernel_spmd(nc, in_maps, core_ids=list(range(NCORES)))
    outs = res.results
    logits = np.concatenate([o["logits"] for o in outs], axis=0)
    w = np.concatenate([o["w"] for o in outs], axis=0)
    return logits.astype(np.float32), w.astype(np.float32)


# revision 18
# speedup vs baseline: 1.1832x; 1.0051x over previous
"""MoE head kernel for Trainium2 (8 NeuronCores, data-parallel over batch).

Per the reference nn.Module:
  w      = softmax(cos_sim(z_cat, mu_cat) / tau)          # gate  [B, E]
  xhat   = LayerNorm(feat)                                 # affine folded into W1/b1
  h_e    = relu(xhat @ W1_e + b1_e)
  l_e    = h_e @ W2_e + b2_e
  logits = sum_e w[:, e] * l_e                             # [B, C]
returns (logits, w).

The LN affine (gamma/beta) is folded into W1/b1 on the host (exact:
x_e @ W1 = xhat @ (gamma*W1) + beta @ W1), so the device kernel has a
single shared xhat for all experts.

Sharding: batch B=16384 split 8 ways (2048 rows/core); params replicated.

Engine streams execute in emission order, so everything that is not the
expert matmul stream (LayerNorm math, xhat transposes, the whole gate)
is interleaved INTO expert 0's loop as small "slot" emissions between
matmul groups — the PE never sits behind a long serial prologue.

Per-core layout (matmul operands bf16 -> FWL weight loads, 1 cyc/row):
  - experts iterate chunk-outer (4 chunks of 512 batch rows), 16 H-tiles
    inner; mm1 accumulates hT [128, 512] over 8 K-tiles in PSUM; relu+bias
    on ScalarE into a persistent h buffer [128, 16, 512] bf16.
  - mm2 runs as a 16-matmul BURST per chunk into one PSUM bank (weight
    loads pipeline within the burst), deferred into the next chunk's
    stream so it never waits on relu.
  - transposes are regular matmuls against an identity (faster than PE
    transpose-mode and they count as PE-busy for the HAM clock gate).
  - gate produces w [B,E] (f32, for output), wT, and a partition-broadcast
    wB[c, e, b] = w[b, e] via tiny selector matmuls.
  - drain per (expert, chunk): logitsT += ps2 * wB[:, e, :] on VectorE
    only.  b2 is pre-accumulated into logitsT via b2.T @ wT matmuls.
  - final transposes back to [B, C] interleave into the tail.
"""

import numpy as np
from contextlib import ExitStack

import ml_dtypes

import concourse.bass as bass
import concourse.mybir as mybir
import concourse.tile as tile
from concourse import bacc
from concourse.masks import make_identity
from concourse.bass_utils import run_bass_kernel_spmd

# Problem shapes (hardcoded per contract).
B, D, H, E, DZ = 16384, 1024, 2048, 8, 256
NCORES = 8
BS = B // NCORES            # rows per core = 2048
CHUNK = 512                 # batch chunk (PSUM bank = 512 fp32)
NCH = BS // CHUNK           # 4
BT = BS // 128              # 16 partition tiles of batch
KD = D // 128               # 8 K-tiles for mm1
MH = H // 128               # 16 M-tiles of hidden
KZ = DZ // 128              # 2 K-tiles for the gate matmul
LN_EPS = 1e-5

F32 = mybir.dt.float32
BF16 = mybir.dt.bfloat16
NPBF = ml_dtypes.bfloat16
AF = mybir.ActivationFunctionType
ALU = mybir.AluOpType
AX = mybir.AxisListType


def _build(tau: float):
    nc = bacc.Bacc(None, target_bir_lowering=False, name="moe_head")

    feat = nc.dram_tensor("feat", [BS, D], F32, kind="ExternalInput")
    z = nc.dram_tensor("z", [BS, DZ], F32, kind="ExternalInput")
    mu = nc.dram_tensor("mu", [E, DZ], F32, kind="ExternalInput")
    # w1 host layout: [e, mt, ki, ko, mi] so each strip DMA is contiguous.
    w1 = nc.dram_tensor("w1", [E, MH, 128, KD, 128], BF16, kind="ExternalInput")
    # w2 host layout: [e, ki, ko, c]
    w2 = nc.dram_tensor("w2", [E, 128, MH, E], BF16, kind="ExternalInput")
    # b1 host layout: [e, mi, mo]
    b1 = nc.dram_tensor("b1", [E, 128, MH], F32, kind="ExternalInput")
    b2 = nc.dram_tensor("b2", [E, E], BF16, kind="ExternalInput")
    sel_d = nc.dram_tensor("sel", [E, E * E], BF16, kind="ExternalInput")
    logits_o = nc.dram_tensor("logits", [BS, E], F32, kind="ExternalOutput")
    w_o = nc.dram_tensor("w", [BS, E], F32, kind="ExternalOutput")

    inv_tau = 1.0 / tau

    with tile.TileContext(nc) as tc, ExitStack() as ctx:
        persist = ctx.enter_context(tc.tile_pool(name="persist", bufs=1))
        ftpool = ctx.enter_context(tc.tile_pool(name="ftp", bufs=5))
        sqpool = ctx.enter_context(tc.tile_pool(name="sqp", bufs=1))
        xhpool = ctx.enter_context(tc.tile_pool(name="xh", bufs=6))
        statp = ctx.enter_context(tc.tile_pool(name="stat", bufs=4))
        wpool = ctx.enter_context(tc.tile_pool(name="w1s", bufs=MH))
        epool = ctx.enter_context(tc.tile_pool(name="eparam", bufs=2))
        spool = ctx.enter_context(tc.tile_pool(name="small", bufs=3))
        psA = ctx.enter_context(tc.tile_pool(name="psA", bufs=2, space="PSUM"))
        psB = ctx.enter_context(tc.tile_pool(name="psB", bufs=2, space="PSUM"))
        psT = ctx.enter_context(tc.tile_pool(name="psT", bufs=4, space="PSUM"))

        # ---- persistent SBUF ----
        xhatT_c = [persist.tile([128, KD, CHUNK], BF16, name=f"xhatT{c}")
                   for c in range(NCH)]
        hall = [persist.tile([128, MH, CHUNK], BF16, name=f"hall{p}")
                for p in range(2)]
        znT = persist.tile([128, KZ, BS], BF16)
        munT = persist.tile([128, KZ, E], BF16)
        wT = persist.tile([E, BS], BF16)          # gate weights, transposed
        wB = persist.tile([E, E, BS], BF16)       # w[b, e] bcast to C partitions
        w_sb = persist.tile([128, BT, E], F32)    # gate weights [B, E]
        accT = persist.tile([E, BS], F32)         # logitsT accumulator
        acc_out = persist.tile([128, BT, E], F32)
        identbf = persist.tile([128, 128], BF16)
        ident8b = persist.tile([E, E], BF16)
        ident8f = persist.tile([E, E], F32)
        sel = persist.tile([E, E * E], BF16)
        b2s = persist.tile([E, E], BF16)
        mu_sb = persist.tile([E, DZ], F32)
        mun_b = persist.tile([E, DZ], BF16)
        eps_sb = persist.tile([128, 1], F32)

        make_identity(nc, identbf)
        make_identity(nc, ident8b)
        make_identity(nc, ident8f)
        nc.vector.memset(eps_sb[:], LN_EPS)

        # activations on sync queue; gate inputs on scalar queue;
        # weights on gpsimd queue (independent DMA streams).
        nc.scalar.dma_start(mu_sb[:], mu[:, :])
        nc.gpsimd.dma_start(b2s[:], b2[:, :])
        nc.gpsimd.dma_start(sel[:], sel_d[:, :])

        # Pre-warm the scalar engine's activation-function tables during the
        # DMA lead-in (lazy table loads otherwise hit the LN critical path).
        warm = persist.tile([128, 1], F32)
        for f in (AF.Copy, AF.Square, AF.Sqrt, AF.Exp, AF.Relu):
            nc.scalar.activation(warm[:], eps_sb[:], f)

        xh_tiles = [None] * BT
        ln_stats = [None] * BT

        def emit_ln_a(bt):
            """LN part A: load + the two row reductions (ScalarE)."""
            bsl = slice(bt * 128, (bt + 1) * 128)
            ft = ftpool.tile([128, D], F32, tag="ft", name=f"ft_{bt}")
            nc.sync.dma_start(ft[:], feat[bsl, :])
            junk = sqpool.tile([128, D], F32, tag="sq", name=f"junk_{bt}")
            s1 = statp.tile([128, 1], F32, tag="s1", name=f"s1_{bt}")
            nc.scalar.activation(junk, ft[:], AF.Copy, accum_out=s1)
            ss = statp.tile([128, 1], F32, tag="ss", name=f"ss_{bt}")
            nc.scalar.activation(junk, ft[:], AF.Square, accum_out=ss)
            ln_stats[bt] = (ft, s1, ss)

        def emit_ln_b(bt):
            """LN part B: stats -> xhat (DVE + one scalar Sqrt)."""
            ft, s1, ss = ln_stats[bt]
            nm = statp.tile([128, 1], F32, tag="nm", name=f"nm_{bt}")
            nc.vector.tensor_scalar_mul(nm, s1, -1.0 / D)
            ms = statp.tile([128, 1], F32, tag="ms", name=f"ms_{bt}")
            nc.vector.tensor_tensor(ms, nm, nm, ALU.mult)
            vv = statp.tile([128, 1], F32, tag="vv", name=f"vv_{bt}")
            nc.vector.scalar_tensor_tensor(vv, ss, 1.0 / D, ms, ALU.mult,
                                           ALU.subtract)
            std = statp.tile([128, 1], F32, tag="std", name=f"std_{bt}")
            nc.scalar.activation(std, vv, AF.Sqrt, bias=eps_sb[:])
            rs = statp.tile([128, 1], F32, tag="rs", name=f"rs_{bt}")
            nc.vector.reciprocal(rs, std)
            xh = xhpool.tile([128, D], BF16, tag="xh", name=f"xh_{bt}")
            nc.vector.tensor_scalar(xh[:], ft[:], nm, rs, ALU.add, ALU.mult)
            xh_tiles[bt] = xh

        def emit_xhat_transpose(c):
            """Transpose this chunk's 4 LN'd tiles into xhatT_c[c]."""
            for j in range(4):
                bt = 4 * c + j
                xh = xh_tiles[bt]
                lo = j * 128
                for g in range(2):
                    tp = psT.tile([128, 4, 128], F32, tag="tp")
                    for jj in range(4):
                        kd = g * 4 + jj
                        nc.tensor.matmul(
                            tp[:, jj, :], xh[:, kd * 128:(kd + 1) * 128],
                            identbf[:], start=True, stop=True)
                    dst = xhatT_c[c][:, g * 4:(g + 1) * 4, lo:lo + 128]
                    if c > 0 and (bt + g) % 2 == 0:
                        nc.scalar.activation(dst, tp[:], AF.Copy)
                    else:
                        nc.vector.tensor_copy(dst, tp[:])

        def emit_mu_norm():
            musq = spool.tile([E, DZ], F32, tag="musq")
            mss = statp.tile([E, 1], F32, tag="mss")
            nc.vector.scalar_tensor_tensor(musq, mu_sb[:], 1.0, mu_sb[:],
                                           ALU.mult, ALU.mult, accum_out=mss)
            mstd = statp.tile([E, 1], F32, tag="mstd")
            nc.scalar.activation(mstd, mss, AF.Sqrt)
            mrn = statp.tile([E, 1], F32, tag="mrn")
            nc.vector.reciprocal(mrn, mstd)
            nc.vector.tensor_scalar_mul(mun_b[:], mu_sb[:], mrn)

        def emit_mu_transpose():
            for kz in range(KZ):
                tpm = psT.tile([128, E], F32, tag="tp")
                nc.tensor.matmul(tpm[:], mun_b[:, kz * 128:(kz + 1) * 128],
                                 ident8b[:], start=True, stop=True)
                nc.vector.tensor_copy(munT[:, kz, :], tpm[:])

        def emit_z(bt):
            """Normalize z rows for one tile + transpose into znT."""
            bsl = slice(bt * 128, (bt + 1) * 128)
            zt = spool.tile([128, DZ], F32, tag="zt", name=f"zt_{bt}")
            nc.scalar.dma_start(zt[:], z[bsl, :])
            zsq = spool.tile([128, DZ], F32, tag="zsq")
            zss = statp.tile([128, 1], F32, tag="zss")
            nc.vector.scalar_tensor_tensor(zsq, zt[:], 1.0, zt[:],
                                           ALU.mult, ALU.mult, accum_out=zss)
            zstd = statp.tile([128, 1], F32, tag="zstd")
            nc.scalar.activation(zstd, zss, AF.Sqrt)
            zrn = statp.tile([128, 1], F32, tag="zrn")
            nc.vector.reciprocal(zrn, zstd)
            znb = spool.tile([128, DZ], BF16, tag="znb")
            nc.vector.tensor_scalar_mul(znb[:], zt[:], zrn)
            tpz = psT.tile([128, KZ, 128], F32, tag="tp")
            for kz in range(KZ):
                nc.tensor.matmul(tpz[:, kz, :], znb[:, kz * 128:(kz + 1) * 128],
                                 identbf[:], start=True, stop=True)
            nc.vector.tensor_copy(znT[:, :, bsl], tpz[:])

        def emit_sims(bt):
            """cos-sims + softmax for one tile -> w_sb row block + wT."""
            bsl = slice(bt * 128, (bt + 1) * 128)
            sps = psT.tile([128, E], F32, tag="tp")
            for kz in range(KZ):
                nc.tensor.matmul(sps[:], znT[:, kz, bsl], munT[:, kz, :],
                                 start=(kz == 0), stop=(kz == KZ - 1))
            ex = spool.tile([128, E], F32, tag="ex")
            if tau >= 0.25:
                # |sims/tau| <= 4: exp cannot overflow; skip max-subtract.
                nc.scalar.activation(ex[:], sps[:], AF.Exp, scale=inv_tau)
            else:
                mx = statp.tile([128, 1], F32, tag="mx")
                nc.vector.tensor_reduce(mx, sps[:], AX.X, ALU.max)
                nb = statp.tile([128, 1], F32, tag="nb")
                nc.vector.tensor_scalar_mul(nb, mx, -inv_tau)
                nc.scalar.activation(ex[:], sps[:], AF.Exp, bias=nb,
                                     scale=inv_tau)
            sm = statp.tile([128, 1], F32, tag="sm")
            nc.vector.tensor_reduce(sm, ex[:], AX.X, ALU.add)
            rsm = statp.tile([128, 1], F32, tag="rsm")
            nc.vector.reciprocal(rsm, sm)
            nc.vector.tensor_scalar_mul(w_sb[:, bt, :], ex[:], rsm)
            wbf = spool.tile([128, E], BF16, tag="wbf")
            nc.vector.tensor_scalar_mul(wbf[:], ex[:], rsm)
            wtp = psT.tile([E, 128], F32, tag="tp")
            nc.tensor.matmul(wtp[:], wbf[:], identbf[:], start=True, stop=True)
            nc.vector.tensor_copy(wT[:, bsl], wtp[:])

        def emit_wb(ch):
            """wB[c, e, b] = w[b, e] for this chunk + accT init with b2."""
            csl = slice(ch * CHUNK, (ch + 1) * CHUNK)
            for e in range(E):
                bc = psT.tile([E, CHUNK], F32, tag="tp")
                nc.tensor.matmul(bc[:], sel[:, e * E:(e + 1) * E], wT[:, csl],
                                 start=True, stop=True)
                if e % 2 == 0:
                    nc.vector.tensor_copy(wB[:, e, csl], bc[:])
                else:
                    nc.scalar.activation(wB[:, e, csl], bc[:], AF.Copy)
            bi = psT.tile([E, CHUNK], F32, tag="tp")
            nc.tensor.matmul(bi[:], b2s[:], wT[:, csl], start=True, stop=True)
            nc.vector.tensor_copy(accT[:, csl], bi[:])

        def slot_cb(c, mt):
            """Gate/LN work interleaved into expert 0's PE stream."""
            if mt in (1, 4, 7, 10):
                if c == 0 and mt == 1:
                    emit_mu_transpose()
                emit_z(4 * c + (mt - 1) // 3)
            if mt in (2, 5, 8, 11) and c < NCH - 1:
                emit_ln_a(4 * (c + 1) + (mt - 2) // 3)
            if mt in (3, 6, 9, 12):
                emit_sims(4 * c + (mt - 3) // 3)
            if mt in (4, 7, 10, 13) and c < NCH - 1:
                emit_ln_b(4 * (c + 1) + (mt - 4) // 3)
            if mt == 14:
                emit_wb(c)
                if c == NCH - 1:
                    nc.sync.dma_start(
                        w_o.rearrange("(bo bi) c -> bi bo c", bi=128), w_sb[:])

        # pending mm2 burst/drain state, flushed inside the next chunk
        pending = []

        def flush_pending():
            if not pending:
                return
            e, c, ps2, hbuf, w2sb = pending.pop()
            for mt in range(MH):
                nc.tensor.matmul(ps2[:], w2sb[:, mt, :], hbuf[:, mt, :],
                                 start=(mt == 0), stop=(mt == MH - 1))
            csl = slice(c * CHUNK, (c + 1) * CHUNK)
            dtmp = spool.tile([E, CHUNK], F32, tag="dtmp")
            nc.vector.tensor_tensor(dtmp[:], ps2[:], wB[:, e, csl], ALU.mult)
            nc.vector.tensor_tensor(accT[:, csl], accT[:, csl], dtmp[:],
                                    ALU.add)
            if e == E - 1:
                for j in range(4):
                    bt = 4 * c + j
                    bsl = slice(bt * 128, (bt + 1) * 128)
                    ltp = psT.tile([128, E], F32, tag="tp")
                    nc.tensor.matmul(ltp[:], accT[:, bsl], ident8f[:],
                                     start=True, stop=True)
                    nc.vector.tensor_copy(acc_out[:, bt, :], ltp[:])
                nc.sync.dma_start(
                    logits_o.rearrange("(bo bi) c -> bi bo c", bi=128)
                    [:, 4 * c:4 * (c + 1), :],
                    acc_out[:, 4 * c:4 * (c + 1), :])

        def emit_expert(e):
            w2sb = epool.tile([128, MH, E], BF16, tag="w2sb",
                              name=f"w2sb_{e}")
            nc.gpsimd.dma_start(w2sb[:], w2[e])
            b1sb = epool.tile([128, MH], F32, tag="b1sb", name=f"b1sb_{e}")
            nc.gpsimd.dma_start(b1sb[:], b1[e])
            strips = [None] * MH
            for c in range(NCH):
                if e == 0:
                    emit_xhat_transpose(c)
                ps2 = psB.tile([E, CHUNK], F32, tag="ps2", name=f"ps2_{e}_{c}")
                hbuf = hall[(e * NCH + c) % 2]
                for mt in range(MH):
                    if e == 0:
                        slot_cb(c, mt)
                    if c == 0:
                        strips[mt] = wpool.tile([128, KD, 128], BF16,
                                                tag="w1s", name=f"w1s_{e}_{mt}")
                        nc.gpsimd.dma_start(strips[mt][:], w1[e, mt])
                    ps1 = psA.tile([128, CHUNK], F32, tag="ps1")
                    for k in range(KD):
                        nc.tensor.matmul(
                            ps1[:], strips[mt][:, k, :], xhatT_c[c][:, k, :],
                            start=(k == 0), stop=(k == KD - 1))
                    nc.scalar.activation(hbuf[:, mt, :], ps1[:], AF.Relu,
                                         bias=b1sb[:, mt:mt + 1])
                    if mt == 0:
                        flush_pending()
                pending.append((e, c, ps2, hbuf, w2sb))

        # prologue: LN for chunk 0's tiles + mu normalization
        emit_ln_a(0)
        emit_ln_a(1)
        emit_ln_b(0)
        emit_ln_a(2)
        emit_ln_b(1)
        emit_ln_a(3)
        emit_ln_b(2)
        emit_ln_b(3)
        emit_mu_norm()

        for e in range(E):
            emit_expert(e)
        flush_pending()

    nc.compile()
    return nc


_CACHE = {}


def _prep_params(inputs):
    """Host-side: fold LN affine into W1/b1, cast+rearrange weights."""
    W1 = np.asarray(inputs["W1"], np.float32)
    b1 = np.asarray(inputs["b1"], np.float32)
    W2 = np.asarray(inputs["W2"], np.float32)
    b2 = np.asarray(inputs["b2"], np.float32)
    gam = np.asarray(inputs["ln_gamma"], np.float32)
    bet = np.asarray(inputs["ln_beta"], np.float32)
    if not np.all(gam == 1.0):
        W1 = W1 * gam[:, :, None]
    if not np.all(bet == 0.0):
        b1 = b1 + np.einsum("ed,edh->eh", bet,
                            np.asarray(inputs["W1"], np.float32))
    w1r = np.ascontiguousarray(
        W1.reshape(E, KD, 128, MH, 128).transpose(0, 3, 2, 1, 4)).astype(NPBF)
    w2r = np.ascontiguousarray(
        W2.reshape(E, MH, 128, E).transpose(0, 2, 1, 3)).astype(NPBF)
    b1r = np.ascontiguousarray(b1.reshape(E, MH, 128).transpose(0, 2, 1))
    b2r = np.ascontiguousarray(b2).astype(NPBF)
    # selector: sel[k, e*E + c] = 1 iff k == e (per-expert row-broadcast)
    selr = np.zeros((E, E * E), NPBF)
    for e in range(E):
        selr[e, e * E:(e + 1) * E] = 1.0
    return w1r, w2r, b1r, b2r, selr


def make_in_maps(inputs):
    feat = np.ascontiguousarray(np.asarray(inputs["feat"], np.float32))
    z_cat = np.ascontiguousarray(np.asarray(inputs["z_cat"], np.float32))
    mu_cat = np.ascontiguousarray(np.asarray(inputs["mu_cat"], np.float32))
    w1r, w2r, b1r, b2r, selr = _prep_params(inputs)
    in_maps = []
    for c in range(NCORES):
        rs = slice(c * BS, (c + 1) * BS)
        in_maps.append({
            "feat": feat[rs],
            "z": z_cat[rs],
            "mu": mu_cat,
            "w1": w1r,
            "w2": w2r,
            "b1": b1r,
            "b2": b2r,
            "sel": selr,
        })
    return in_maps


def kernel(**inputs):
    tau = max(1e-6, float(np.asarray(inputs["tau_gate"])))
    key = (tau,)
    if key not in _CACHE:
        _CACHE[key] = _build(tau)
    nc = _CACHE[key]

    in_maps = make_in_maps(inputs)
    res = run_bass_kernel_spmd(nc, in_maps, core_ids=list(range(NCORES)))
    outs = res.results
    logits = np.concatenate([o["logits"] for o in outs], axis=0)
    w = np.concatenate([o["w"] for o in outs], axis=0)
    return logits.astype(np.float32), w.astype(np.float32)


# revision 23
# speedup vs baseline: 1.1837x; 1.0005x over previous
"""MoE head kernel for Trainium2 (8 NeuronCores, data-parallel over batch).

Per the reference nn.Module:
  w      = softmax(cos_sim(z_cat, mu_cat) / tau)          # gate  [B, E]
  xhat   = LayerNorm(feat)                                 # affine folded into W1/b1
  h_e    = relu(xhat @ W1_e + b1_e)
  l_e    = h_e @ W2_e + b2_e
  logits = sum_e w[:, e] * l_e                             # [B, C]
returns (logits, w).

The LN affine (gamma/beta) is folded into W1/b1 on the host (exact:
x_e @ W1 = xhat @ (gamma*W1) + beta @ W1), so the device kernel has a
single shared xhat for all experts.

Sharding: batch B=16384 split 8 ways (2048 rows/core); params replicated.

Engine streams execute in emission order, so everything that is not the
expert matmul stream (LayerNorm math, xhat transposes, the whole gate)
is interleaved INTO expert 0's loop as small "slot" emissions between
matmul groups — the PE never sits behind a long serial prologue.

Per-core layout (matmul operands bf16 -> FWL weight loads, 1 cyc/row):
  - experts iterate chunk-outer (4 chunks of 512 batch rows), 16 H-tiles
    inner; mm1 accumulates hT [128, 512] over 8 K-tiles in PSUM; relu+bias
    on ScalarE into a persistent h buffer [128, 16, 512] bf16.
  - mm2 runs as a 16-matmul BURST per chunk into one PSUM bank (weight
    loads pipeline within the burst), deferred into the next chunk's
    stream so it never waits on relu.
  - transposes are regular matmuls against an identity (faster than PE
    transpose-mode and they count as PE-busy for the HAM clock gate).
  - gate produces w [B,E] (f32, for output), wT, and a partition-broadcast
    wB[c, e, b] = w[b, e] via tiny selector matmuls.
  - drain per (expert, chunk): logitsT += ps2 * wB[:, e, :] on VectorE
    only.  b2 is pre-accumulated into logitsT via b2.T @ wT matmuls.
  - final transposes back to [B, C] interleave into the tail.
"""

import numpy as np
from contextlib import ExitStack

import ml_dtypes

import concourse.bass as bass
import concourse.mybir as mybir
import concourse.tile as tile
from concourse import bacc
from concourse.masks import make_identity
from concourse.bass_utils import run_bass_kernel_spmd

# Problem shapes (hardcoded per contract).
B, D, H, E, DZ = 16384, 1024, 2048, 8, 256
NCORES = 8
BS = B // NCORES            # rows per core = 2048
CHUNK = 512                 # batch chunk (PSUM bank = 512 fp32)
NCH = BS // CHUNK           # 4
BT = BS // 128              # 16 partition tiles of batch
KD = D // 128               # 8 K-tiles for mm1
MH = H // 128               # 16 M-tiles of hidden
KZ = DZ // 128              # 2 K-tiles for the gate matmul
LN_EPS = 1e-5

F32 = mybir.dt.float32
BF16 = mybir.dt.bfloat16
NPBF = ml_dtypes.bfloat16
AF = mybir.ActivationFunctionType
ALU = mybir.AluOpType
AX = mybir.AxisListType


def _build(tau: float):
    nc = bacc.Bacc(None, target_bir_lowering=False, name="moe_head")

    feat = nc.dram_tensor("feat", [BS, D], F32, kind="ExternalInput")
    z = nc.dram_tensor("z", [BS, DZ], F32, kind="ExternalInput")
    mu = nc.dram_tensor("mu", [E, DZ], F32, kind="ExternalInput")
    # w1 host layout: [e, mt, ki, ko, mi] so each strip DMA is contiguous.
    w1 = nc.dram_tensor("w1", [E, MH, 128, KD, 128], BF16, kind="ExternalInput")
    # w2 host layout: [e, ki, ko, c]
    w2 = nc.dram_tensor("w2", [E, 128, MH, E], BF16, kind="ExternalInput")
    # b1 host layout: [e, mi, mo]
    b1 = nc.dram_tensor("b1", [E, 128, MH], F32, kind="ExternalInput")
    b2 = nc.dram_tensor("b2", [E, E], BF16, kind="ExternalInput")
    sel_d = nc.dram_tensor("sel", [E, E * E], BF16, kind="ExternalInput")
    logits_o = nc.dram_tensor("logits", [BS, E], F32, kind="ExternalOutput")
    w_o = nc.dram_tensor("w", [BS, E], F32, kind="ExternalOutput")

    inv_tau = 1.0 / tau

    with tile.TileContext(nc) as tc, ExitStack() as ctx:
        persist = ctx.enter_context(tc.tile_pool(name="persist", bufs=1))
        ftpool = ctx.enter_context(tc.tile_pool(name="ftp", bufs=5))
        sqpool = ctx.enter_context(tc.tile_pool(name="sqp", bufs=1))
        xhpool = ctx.enter_context(tc.tile_pool(name="xh", bufs=6))
        statp = ctx.enter_context(tc.tile_pool(name="stat", bufs=4))
        wpool = ctx.enter_context(tc.tile_pool(name="w1s", bufs=MH))
        epool = ctx.enter_context(tc.tile_pool(name="eparam", bufs=2))
        spool = ctx.enter_context(tc.tile_pool(name="small", bufs=3))
        psA = ctx.enter_context(tc.tile_pool(name="psA", bufs=2, space="PSUM"))
        psB = ctx.enter_context(tc.tile_pool(name="psB", bufs=2, space="PSUM"))
        psT = ctx.enter_context(tc.tile_pool(name="psT", bufs=4, space="PSUM"))

        # ---- persistent SBUF ----
        xhatT_c = [persist.tile([128, KD, CHUNK], BF16, name=f"xhatT{c}")
                   for c in range(NCH)]
        hall = [persist.tile([128, MH, CHUNK], BF16, name=f"hall{p}")
                for p in range(2)]
        znT = persist.tile([128, KZ, BS], BF16)
        munT = persist.tile([128, KZ, E], BF16)
        wT = persist.tile([E, BS], BF16)          # gate weights, transposed
        wB = persist.tile([E, E, BS], BF16)       # w[b, e] bcast to C partitions
        w_sb = persist.tile([128, BT, E], F32)    # gate weights [B, E]
        accT = persist.tile([E, BS], F32)         # logitsT accumulator
        acc_out = persist.tile([128, BT, E], F32)
        identbf = persist.tile([128, 128], BF16)
        ident8b = persist.tile([E, E], BF16)
        ident8f = persist.tile([E, E], F32)
        sel = persist.tile([E, E * E], BF16)
        b2s = persist.tile([E, E], BF16)
        mu_sb = persist.tile([E, DZ], F32)
        mun_b = persist.tile([E, DZ], BF16)
        eps_sb = persist.tile([128, 1], F32)

        make_identity(nc, identbf)
        make_identity(nc, ident8b)
        make_identity(nc, ident8f)
        nc.vector.memset(eps_sb[:], LN_EPS)

        # activations on sync queue; gate inputs on scalar queue;
        # weights on gpsimd queue (independent DMA streams).
        nc.scalar.dma_start(mu_sb[:], mu[:, :])
        nc.gpsimd.dma_start(b2s[:], b2[:, :])
        nc.gpsimd.dma_start(sel[:], sel_d[:, :])

        # Pre-warm activation-function tables the prologue doesn't use
        # (lazy table loads would otherwise hit the gate/relu critical path).
        warm = persist.tile([128, 1], F32)
        for f in (AF.Sqrt, AF.Exp, AF.Relu):
            nc.scalar.activation(warm[:], eps_sb[:], f)

        xh_tiles = [None] * BT
        ln_stats = [None] * BT

        def emit_ln_a(bt):
            """LN part A: load + the two row reductions (ScalarE)."""
            bsl = slice(bt * 128, (bt + 1) * 128)
            ft = ftpool.tile([128, D], F32, tag="ft", name=f"ft_{bt}")
            nc.sync.dma_start(ft[:], feat[bsl, :])
            junk = sqpool.tile([128, D], F32, tag="sq", name=f"junk_{bt}")
            s1 = statp.tile([128, 1], F32, tag="s1", name=f"s1_{bt}")
            nc.scalar.activation(junk, ft[:], AF.Copy, accum_out=s1)
            ss = statp.tile([128, 1], F32, tag="ss", name=f"ss_{bt}")
            nc.scalar.activation(junk, ft[:], AF.Square, accum_out=ss)
            ln_stats[bt] = (ft, s1, ss)

        def emit_ln_b(bt):
            """LN part B: stats -> xhat (DVE + one scalar Sqrt)."""
            ft, s1, ss = ln_stats[bt]
            nm = statp.tile([128, 1], F32, tag="nm", name=f"nm_{bt}")
            nc.vector.tensor_scalar_mul(nm, s1, -1.0 / D)
            ms = statp.tile([128, 1], F32, tag="ms", name=f"ms_{bt}")
            nc.vector.tensor_tensor(ms, nm, nm, ALU.mult)
            vv = statp.tile([128, 1], F32, tag="vv", name=f"vv_{bt}")
            nc.vector.scalar_tensor_tensor(vv, ss, 1.0 / D, ms, ALU.mult,
                                           ALU.subtract)
            std = statp.tile([128, 1], F32, tag="std", name=f"std_{bt}")
            nc.scalar.activation(std, vv, AF.Sqrt, bias=eps_sb[:])
            rs = statp.tile([128, 1], F32, tag="rs", name=f"rs_{bt}")
            nc.vector.reciprocal(rs, std)
            xh = xhpool.tile([128, D], BF16, tag="xh", name=f"xh_{bt}")
            nc.vector.tensor_scalar(xh[:], ft[:], nm, rs, ALU.add, ALU.mult)
            xh_tiles[bt] = xh

        def emit_xhat_transpose(c):
            """Transpose this chunk's 4 LN'd tiles into xhatT_c[c]."""
            for j in range(4):
                bt = 4 * c + j
                xh = xh_tiles[bt]
                lo = j * 128
                for g in range(2):
                    tp = psT.tile([128, 4, 128], F32, tag="tp")
                    for jj in range(4):
                        kd = g * 4 + jj
                        nc.tensor.matmul(
                            tp[:, jj, :], xh[:, kd * 128:(kd + 1) * 128],
                            identbf[:], start=True, stop=True)
                    dst = xhatT_c[c][:, g * 4:(g + 1) * 4, lo:lo + 128]
                    if c > 0 and (bt + g) % 2 == 0:
                        nc.scalar.activation(dst, tp[:], AF.Copy)
                    else:
                        nc.vector.tensor_copy(dst, tp[:])

        def emit_mu_norm():
            musq = spool.tile([E, DZ], F32, tag="musq")
            mss = statp.tile([E, 1], F32, tag="mss")
            nc.vector.scalar_tensor_tensor(musq, mu_sb[:], 1.0, mu_sb[:],
                                           ALU.mult, ALU.mult, accum_out=mss)
            mstd = statp.tile([E, 1], F32, tag="mstd")
            nc.scalar.activation(mstd, mss, AF.Sqrt)
            mrn = statp.tile([E, 1], F32, tag="mrn")
            nc.vector.reciprocal(mrn, mstd)
            nc.vector.tensor_scalar_mul(mun_b[:], mu_sb[:], mrn)

        def emit_mu_transpose():
            for kz in range(KZ):
                tpm = psT.tile([128, E], F32, tag="tp")
                nc.tensor.matmul(tpm[:], mun_b[:, kz * 128:(kz + 1) * 128],
                                 ident8b[:], start=True, stop=True)
                nc.vector.tensor_copy(munT[:, kz, :], tpm[:])

        def emit_z(bt):
            """Normalize z rows for one tile + transpose into znT."""
            bsl = slice(bt * 128, (bt + 1) * 128)
            zt = spool.tile([128, DZ], F32, tag="zt", name=f"zt_{bt}")
            nc.scalar.dma_start(zt[:], z[bsl, :])
            zsq = spool.tile([128, DZ], F32, tag="zsq")
            zss = statp.tile([128, 1], F32, tag="zss")
            nc.vector.scalar_tensor_tensor(zsq, zt[:], 1.0, zt[:],
                                           ALU.mult, ALU.mult, accum_out=zss)
            zstd = statp.tile([128, 1], F32, tag="zstd")
            nc.scalar.activation(zstd, zss, AF.Sqrt)
            zrn = statp.tile([128, 1], F32, tag="zrn")
            nc.vector.reciprocal(zrn, zstd)
            znb = spool.tile([128, DZ], BF16, tag="znb")
            nc.vector.tensor_scalar_mul(znb[:], zt[:], zrn)
            tpz = psT.tile([128, KZ, 128], F32, tag="tp")
            for kz in range(KZ):
                nc.tensor.matmul(tpz[:, kz, :], znb[:, kz * 128:(kz + 1) * 128],
                                 identbf[:], start=True, stop=True)
            nc.vector.tensor_copy(znT[:, :, bsl], tpz[:])

        def emit_sims(bt):
            """cos-sims + softmax for one tile -> w_sb row block + wT."""
            bsl = slice(bt * 128, (bt + 1) * 128)
            sps = psT.tile([128, E], F32, tag="tp")
            for kz in range(KZ):
                nc.tensor.matmul(sps[:], znT[:, kz, bsl], munT[:, kz, :],
                                 start=(kz == 0), stop=(kz == KZ - 1))
            ex = spool.tile([128, E], F32, tag="ex")
            if tau >= 0.25:
                # |sims/tau| <= 4: exp cannot overflow; skip max-subtract.
                nc.scalar.activation(ex[:], sps[:], AF.Exp, scale=inv_tau)
            else:
                mx = statp.tile([128, 1], F32, tag="mx")
                nc.vector.tensor_reduce(mx, sps[:], AX.X, ALU.max)
                nb = statp.tile([128, 1], F32, tag="nb")
                nc.vector.tensor_scalar_mul(nb, mx, -inv_tau)
                nc.scalar.activation(ex[:], sps[:], AF.Exp, bias=nb,
                                     scale=inv_tau)
            sm = statp.tile([128, 1], F32, tag="sm")
            nc.vector.tensor_reduce(sm, ex[:], AX.X, ALU.add)
            rsm = statp.tile([128, 1], F32, tag="rsm")
            nc.vector.reciprocal(rsm, sm)
            nc.vector.tensor_scalar_mul(w_sb[:, bt, :], ex[:], rsm)
            wbf = spool.tile([128, E], BF16, tag="wbf")
            nc.vector.tensor_scalar_mul(wbf[:], ex[:], rsm)
            wtp = psT.tile([E, 128], F32, tag="tp")
            nc.tensor.matmul(wtp[:], wbf[:], identbf[:], start=True, stop=True)
            nc.vector.tensor_copy(wT[:, bsl], wtp[:])

        def emit_wb(ch):
            """wB[c, e, b] = w[b, e] for this chunk + accT init with b2."""
            csl = slice(ch * CHUNK, (ch + 1) * CHUNK)
            for e in range(E):
                bc = psT.tile([E, CHUNK], F32, tag="tp")
                nc.tensor.matmul(bc[:], sel[:, e * E:(e + 1) * E], wT[:, csl],
                                 start=True, stop=True)
                if e % 2 == 0:
                    nc.vector.tensor_copy(wB[:, e, csl], bc[:])
                else:
                    nc.scalar.activation(wB[:, e, csl], bc[:], AF.Copy)
            bi = psT.tile([E, CHUNK], F32, tag="tp")
            nc.tensor.matmul(bi[:], b2s[:], wT[:, csl], start=True, stop=True)
            nc.vector.tensor_copy(accT[:, csl], bi[:])

        def slot_cb(c, mt):
            """Gate/LN work interleaved into expert 0's PE stream."""
            if c == 0:
                # chunk 0: run the gate chains first (clean scalar queue for
                # exp), LN for chunk 1 later in the chunk.
                if mt == 1:
                    emit_mu_transpose()
                if mt in (1, 2, 3, 4):
                    emit_z(mt - 1)
                if mt in (5, 6, 7, 8):
                    emit_sims(mt - 5)
                if mt in (7, 9, 11, 13):
                    emit_ln_a(4 + (mt - 7) // 2)
                if mt in (9, 11, 13, 15):
                    emit_ln_b(4 + (mt - 9) // 2)
            else:
                if mt in (1, 4, 7, 10):
                    emit_z(4 * c + (mt - 1) // 3)
                if mt in (2, 5, 8, 11) and c < NCH - 1:
                    emit_ln_a(4 * (c + 1) + (mt - 2) // 3)
                if mt in (3, 6, 9, 12):
                    emit_sims(4 * c + (mt - 3) // 3)
                if mt in (4, 7, 10, 13) and c < NCH - 1:
                    emit_ln_b(4 * (c + 1) + (mt - 4) // 3)
            if mt == 14:
                emit_wb(c)
                if c == NCH - 1:
                    nc.sync.dma_start(
                        w_o.rearrange("(bo bi) c -> bi bo c", bi=128), w_sb[:])

        # pending mm2 burst/drain state, flushed inside the next chunk
        pending = []
        burst_done = [0]

        def flush_pending():
            if not pending:
                return
            e, c, ps2, hbuf, w2sb, mt0 = pending.pop()
            for mt in range(mt0, MH):
                nc.tensor.matmul(ps2[:], w2sb[:, mt, :], hbuf[:, mt, :],
                                 start=(mt == 0), stop=(mt == MH - 1))
            csl = slice(c * CHUNK, (c + 1) * CHUNK)
            dtmp = spool.tile([E, CHUNK], F32, tag="dtmp")
            nc.vector.tensor_tensor(dtmp[:], ps2[:], wB[:, e, csl], ALU.mult)
            nc.vector.tensor_tensor(accT[:, csl], accT[:, csl], dtmp[:],
                                    ALU.add)
            if e == E - 1:
                for j in range(4):
                    bt = 4 * c + j
                    bsl = slice(bt * 128, (bt + 1) * 128)
                    ltp = psT.tile([128, E], F32, tag="tp")
                    nc.tensor.matmul(ltp[:], accT[:, bsl], ident8f[:],
                                     start=True, stop=True)
                    nc.vector.tensor_copy(acc_out[:, bt, :], ltp[:])
                nc.sync.dma_start(
                    logits_o.rearrange("(bo bi) c -> bi bo c", bi=128)
                    [:, 4 * c:4 * (c + 1), :],
                    acc_out[:, 4 * c:4 * (c + 1), :])

        def emit_expert(e):
            w2sb = epool.tile([128, MH, E], BF16, tag="w2sb",
                              name=f"w2sb_{e}")
            nc.gpsimd.dma_start(w2sb[:], w2[e])
            b1sb = epool.tile([128, MH], F32, tag="b1sb", name=f"b1sb_{e}")
            nc.gpsimd.dma_start(b1sb[:], b1[e])
            strips = [None] * MH
            for c in range(NCH):
                if e == 0:
                    emit_xhat_transpose(c)
                ps2 = psB.tile([E, CHUNK], F32, tag="ps2", name=f"ps2_{e}_{c}")
                hbuf = hall[(e * NCH + c) % 2]
                for mt in range(MH):
                    if e == 0:
                        slot_cb(c, mt)
                    if c == 0:
                        strips[mt] = wpool.tile([128, KD, 128], BF16,
                                                tag="w1s", name=f"w1s_{e}_{mt}")
                        nc.gpsimd.dma_start(strips[mt][:], w1[e, mt])
                    ps1 = psA.tile([128, CHUNK], F32, tag="ps1")
                    for k in range(KD):
                        nc.tensor.matmul(
                            ps1[:], strips[mt][:, k, :], xhatT_c[c][:, k, :],
                            start=(k == 0), stop=(k == KD - 1))
                    nc.scalar.activation(hbuf[:, mt, :], ps1[:], AF.Relu,
                                         bias=b1sb[:, mt:mt + 1])
                    if mt == 0:
                        flush_pending()
                    if e == E - 1 and c == NCH - 1 and mt == 9:
                        # half-flush the last chunk's mm2 early to cut the
                        # end-of-kernel tail
                        for m2 in range(MH // 2):
                            nc.tensor.matmul(
                                ps2[:], w2sb[:, m2, :], hbuf[:, m2, :],
                                start=(m2 == 0), stop=False)
                        burst_done[0] = MH // 2
                pending.append((e, c, ps2, hbuf, w2sb, burst_done[0]))
                burst_done[0] = 0

        # prologue: LN for chunk 0's tiles + mu normalization
        emit_ln_a(0)
        emit_ln_a(1)
        emit_ln_b(0)
        emit_ln_a(2)
        emit_ln_b(1)
        emit_ln_a(3)
        emit_ln_b(2)
        emit_ln_b(3)
        emit_mu_norm()

        for e in range(E):
            emit_expert(e)
        flush_pending()

    nc.compile()
    return nc


_CACHE = {}


def _prep_params(inputs):
    """Host-side: fold LN affine into W1/b1, cast+rearrange weights."""
    W1 = np.asarray(inputs["W1"], np.float32)
    b1 = np.asarray(inputs["b1"], np.float32)
    W2 = np.asarray(inputs["W2"], np.float32)
    b2 = np.asarray(inputs["b2"], np.float32)
    gam = np.asarray(inputs["ln_gamma"], np.float32)
    bet = np.asarray(inputs["ln_beta"], np.float32)
    if not np.all(gam == 1.0):
        W1 = W1 * gam[:, :, None]
    if not np.all(bet == 0.0):
        b1 = b1 + np.einsum("ed,edh->eh", bet,
                            np.asarray(inputs["W1"], np.float32))
    w1r = np.ascontiguousarray(
        W1.reshape(E, KD, 128, MH, 128).transpose(0, 3, 2, 1, 4)).astype(NPBF)
    w2r = np.ascontiguousarray(
        W2.reshape(E, MH, 128, E).transpose(0, 2, 1, 3)).astype(NPBF)
    b1r = np.ascontiguousarray(b1.reshape(E, MH, 128).transpose(0, 2, 1))
    b2r = np.ascontiguousarray(b2).astype(NPBF)
    # selector: sel[k, e*E + c] = 1 iff k == e (per-expert row-broadcast)
    selr = np.zeros((E, E * E), NPBF)
    for e in range(E):
        selr[e, e * E:(e + 1) * E] = 1.0
    return w1r, w2r, b1r, b2r, selr


def make_in_maps(inputs):
    feat = np.ascontiguousarray(np.asarray(inputs["feat"], np.float32))
    z_cat = np.ascontiguousarray(np.asarray(inputs["z_cat"], np.float32))
    mu_cat = np.ascontiguousarray(np.asarray(inputs["mu_cat"], np.float32))
    w1r, w2r, b1r, b2r, selr = _prep_params(inputs)
    in_maps = []
    for c in range(NCORES):
        rs = slice(c * BS, (c + 1) * BS)
        in_maps.append({
            "feat": feat[rs],
            "z": z_cat[rs],
            "mu": mu_cat,
            "w1": w1r,
            "w2": w2r,
            "b1": b1r,
            "b2": b2r,
            "sel": selr,
        })
    return in_maps


def kernel(**inputs):
    tau = max(1e-6, float(np.asarray(inputs["tau_gate"])))
    key = (tau,)
    if key not in _CACHE:
        _CACHE[key] = _build(tau)
    nc = _CACHE[key]

    in_maps = make_in_maps(inputs)
    res = run_bass_kernel_spmd(nc, in_maps, core_ids=list(range(NCORES)))
    outs = res.results
    logits = np.concatenate([o["logits"] for o in outs], axis=0)
    w = np.concatenate([o["w"] for o in outs], axis=0)
    return logits.astype(np.float32), w.astype(np.float32)
